# revision 15
# baseline (speedup 1.0000x reference)
"""Adaptive-softmax loss (nn_AdaptiveLoss) on 8 trn2 NeuronCores.

Strategy: tensor-parallel over the vocab dimension, 8-way. Each core owns
1/8 of the shortlist head columns and 1/8 of each tail cluster's output
rows. Per core:

  - computes cluster hidden states h_g = x @ proj_g.T (replicated, small)
    with fp8 DoubleRow matmuls; h0/h1 requantized to fp8, h2/h3 to bf16,
  - computes its slice of every group's logits: fp8 DoubleRow for the
    K>=256 groups (head/c0/c1), bf16 for the K<=128 clusters (c2/c3);
    weights are prescaled so every group's PSUM logit carries the same
    x64 factor, folded back out via the exp activation scale,
  - exp()s the logits on ACT in 7 PSUM pieces per 128-row tile, with the
    ACT accumulator giving per-piece sums; per-group softmax denominators
    are reconstructed from piece sums plus four narrow boundary sums on
    DVE,
  - gathers exp(logit) at this core's share of the targets straight out
    of SBUF (gpsimd indirect_copy, kept awake by dummy gathers before the
    final row tile) and takes ln in-loop,
  - per-row weight sums (den, W_g) are input-only quantities computed on
    the host and shipped as small tensors - they never ride a collective,
  - the per-row partial stats (5 softmax Zs + weighted-logit numerator,
    [128, 48] fp32) are exchanged with the 7 peers by direct remote SBUF
    DMA writes (XOR-slot all-gather, descriptors generated early and
    triggered at the end), then every core sums the 8 slots and finishes
    the cheap log/normalize arithmetic identically.

The full [B, VOCAB] log-prob matrix is never materialized anywhere, and
no ncfw collective is on the critical path (only the kernel-entry
barrier AllGather, which completes long before the tail needs it).
"""

import sys

sys.path.insert(0, "/opt/trn_rl_repo")

from contextlib import ExitStack

import ml_dtypes
import numpy as np

import concourse.bass as bass  # noqa: F401  (engine types via nc.*)
import concourse.mybir as mybir
import concourse.tile as tile
from concourse import bacc
from concourse.bass_utils import run_bass_kernel_spmd

BF16 = ml_dtypes.bfloat16
F8 = ml_dtypes.float8_e4m3
F32 = mybir.dt.float32
BF16_DT = mybir.dt.bfloat16
F8_DT = mybir.dt.float8e4
U16 = mybir.dt.uint16

NCORES = 8
B, T, D = 1024, 128, 1024
VOCAB, SHORT = 100000, 10000
CL_SIZES = [10000, 20000, 40000, 20000]
CL_D = [512, 256, 128, 64]
SH_SHARD = SHORT // NCORES                      # 1250
CL_SHARD = [s // NCORES for s in CL_SIZES]      # 1250 2500 5000 2500
GRP_BOUNDS = [0, 10000, 20000, 40000, 80000, 100000]
GRP_SHARD = [SH_SHARD] + CL_SHARD

# per-core concatenated logits layout: [head | links(4) | c0 | c1 | c2 | c3]
OFF_HEAD = 0
OFF_LINK = SH_SHARD                              # 1250
OFF_CL = [1254, 2504, 5004, 10004]
GRP_OFF = [OFF_HEAD] + OFF_CL                    # per-group concat offset
CONCAT = OFF_CL[-1] + CL_SHARD[-1]               # 12504
CONCAT_PAD = 12544
# pad slots gather column 0 (always computed, finite); their wm==0 makes
# the contribution vanish.
PADIDX = 0
RT = 8                                           # row tiles of 128

# fp8 scale factors (folded back out via the exp activation scale)
S_WHEAD = 64.0                                   # head weight prescale
S_PROJ = 32.0                                    # proj prescale -> h scale
S_WOUT = 2.0                                     # c0/c1 out-proj prescale
S_W23 = 2.0                                      # c2/c3 out-proj prescale
EXP_TABLE_ID = 6                                 # natural_log_exp_and_others
import os as _os
FP8_ON = not _os.environ.get("ADAK_BF16")

# PSUM piece bounds: head split in two 1024 pieces so its PE-heavy matmuls
# hide behind 2048-wide cluster exps (2-slot PSUM pipeline)
PB = [0, 1024, 2048, 4096, 6144, 8192, 10240, 12288, CONCAT]
NPIECE = 8
# (lo, hi) of the boundary small-side sums, their sv slot = index
SVS = [(1024, 1250), (2048, 2504), (4096, 5004), (10004, 10240)]
# payload stats per row: q = 0 Zh, 1..4 Zc_g, 5 numraw
NSTAT = 6
PAYW = NSTAT * RT                                # 48


# ----------------------------------------------------------------------------
# device kernel builder
# ----------------------------------------------------------------------------

_CACHE: dict[int, object] = {}


def _build(S: int):
    """Build + compile the SPMD kernel for padded slot count S (multiple of 16)."""
    if S in _CACHE:
        return _CACHE[S]
    SW = S // 16

    nc = bacc.Bacc("TRN2", target_bir_lowering=False, debug=False,
                   num_devices=NCORES)

    MMDT = F8_DT if FP8_ON else BF16_DT
    xt_d = nc.dram_tensor("xt", [D, B], MMDT, kind="ExternalInput")
    projt_d = nc.dram_tensor("projt", [D, sum(CL_D)], MMDT, kind="ExternalInput")
    whead_d = nc.dram_tensor("wheadt", [D, 1254], MMDT, kind="ExternalInput")
    wout0_d = nc.dram_tensor("wout0t", [CL_D[0], CL_SHARD[0]], MMDT,
                             kind="ExternalInput")
    wout1_d = nc.dram_tensor("wout1t", [CL_D[1], CL_SHARD[1]], MMDT,
                             kind="ExternalInput")
    wout2_d = nc.dram_tensor("wout2t", [CL_D[2], CL_SHARD[2]], BF16_DT,
                             kind="ExternalInput")
    wout3_d = nc.dram_tensor("wout3t", [CL_D[3], CL_SHARD[3]], BF16_DT,
                             kind="ExternalInput")
    tix_d = nc.dram_tensor("tgtidx", [128, RT * SW], U16, kind="ExternalInput")
    wm_d = nc.dram_tensor("wm", [128, RT, S], BF16_DT, kind="ExternalInput")
    den_d = nc.dram_tensor("den", [128, RT], F32, kind="ExternalInput")
    rden_d = nc.dram_tensor("rden", [128, RT], F32, kind="ExternalInput")
    wgq_d = nc.dram_tensor("wgq", [128, 4, RT], F32, kind="ExternalInput")
    out_d = nc.dram_tensor("out", [1, 1], F32, kind="ExternalOutput")
    DBG = bool(_os.environ.get("ADAK_DBG"))
    if DBG:
        pay_d = nc.dram_tensor("pay_dump", [128, PAYW], F32,
                               kind="ExternalOutput")
        rsum_d = nc.dram_tensor("rsum_dump", [128, PAYW], F32,
                                kind="ExternalOutput")
        zcomb_d = nc.dram_tensor("zcomb_dump", [128, 40], F32,
                                 kind="ExternalOutput")

    EXP = mybir.ActivationFunctionType.Exp
    LN = mybir.ActivationFunctionType.Ln
    ADD = mybir.AluOpType.add
    SUB = mybir.AluOpType.subtract
    MULT = mybir.AluOpType.mult
    AXX = mybir.AxisListType.X
    DR = mybir.MatmulPerfMode.DoubleRow

    with tile.TileContext(nc) as tc, ExitStack() as ctx:
        sb = ctx.enter_context(tc.tile_pool(name="sb", bufs=1))
        big = ctx.enter_context(tc.tile_pool(name="big", bufs=3))
        ps = ctx.enter_context(tc.tile_pool(name="ps", bufs=2, space="PSUM"))

        # combined exp+ln activation table so EXP and LN interleave with a
        # single table load for the whole kernel
        import os
        if not os.environ.get("ADAK_NO_TABLE_PRELOAD"):
            nc.scalar.add_instruction(mybir.InstLoadActFuncSet(
                name=nc.get_next_instruction_name(),
                act_func_set_id=EXP_TABLE_ID, ins=[], outs=[]))

        # ---- persistent SBUF tensors ----
        xt_sb = sb.tile([128, 8, B], MMDT)             # x.T  [d, b] k-tiled
        whead_sb = sb.tile([128, 8, 1254], MMDT)
        wout0_sb = sb.tile([128, 4, CL_SHARD[0]], MMDT)
        wout1_sb = sb.tile([128, 2, CL_SHARD[1]], MMDT)
        wout2_sb = sb.tile([128, CL_SHARD[2]], BF16_DT)
        wout3_sb = sb.tile([64, CL_SHARD[3]], BF16_DT)
        h0_sb = sb.tile([128, 4, B], MMDT)             # h.T (x S_PROJ)
        h1_sb = sb.tile([128, 2, B], MMDT)
        h2_sb = sb.tile([128, B], BF16_DT)
        h3_sb = sb.tile([64, B], BF16_DT)
        tix_sb = sb.tile([128, RT * SW], U16)
        vg3 = sb.tile([128, RT, S], BF16_DT)           # gathered exp(logit)
        wm_sb = sb.tile([128, RT, S], BF16_DT)         # (1-dp)*ownership
        logv3 = sb.tile([128, RT, S], BF16_DT)
        llinkraw = sb.tile([128, RT, 4], F32)          # raw link logits
        zscr = sb.tile([128, 2048], BF16_DT)
        zs = sb.tile([128, RT, NPIECE], F32)  # per-piece exp-sum accumulators
        sv = sb.tile([128, RT, 4], F32)       # boundary small-side sums
        pay = sb.tile([128, PAYW], F32)       # per-core stats payload
        rgath = sb.tile([128, 8, PAYW], F32)  # peer payload gather slots
        rsum = sb.tile([128, PAYW], F32)
        den_sb = sb.tile([128, RT], F32)
        rden_sb = sb.tile([128, RT], F32)
        wgq_sb = sb.tile([128, 4, RT], F32)
        ones_sb = sb.tile([128, 1], F32)
        warmg = sb.tile([128, 16], BF16_DT)   # gpsimd keep-awake gather dst
        zix = sb.tile([128, 1], U16)
        out_sb = sb.tile([1, 1], F32)

        pview = pay[:, :].rearrange("p (q r) -> p q r", q=NSTAT)
        rsq = rsum[:, :].rearrange("p (q r) -> p q r", q=NSTAT)

        # ---- remote all-gather plumbing: clear the handshake sems before
        # any peer can possibly send (their sends sit behind the kernel
        # entry barrier + ~160us of compute) ----
        rsem = nc.alloc_semaphore("adak_rsem")
        lsem = nc.alloc_semaphore("adak_lsem")
        psem = nc.alloc_semaphore("adak_psem")
        with tc.tile_critical():
            nc.gpsimd.sem_clear(rsem)
            nc.gpsimd.sem_clear(lsem)
            nc.gpsimd.sem_clear(psem)

        # ---- input DMAs (order matters: compute-critical tensors first;
        # xt/projt interleaved per k-tile so the h matmuls start early) ----
        pj = sb.tile([128, 8, sum(CL_D)], MMDT)
        xt_r = xt_d.ap().rearrange("(k p) b -> p k b", p=128)
        pj_r = projt_d.ap().rearrange("(k p) c -> p k c", p=128)
        wh_r = whead_d.ap().rearrange("(k p) c -> p k c", p=128)
        for k in range(8):
            nc.sync.dma_start(out=xt_sb[:, k, :], in_=xt_r[:, k, :])
            nc.sync.dma_start(out=whead_sb[:, k, :], in_=wh_r[:, k, :])
        for k in range(8):
            nc.sync.dma_start(out=pj[:, k, :], in_=pj_r[:, k, :])
        nc.sync.dma_start(out=wout0_sb,
                          in_=wout0_d.ap().rearrange("(k p) c -> p k c", p=128))
        nc.sync.dma_start(out=wout1_sb,
                          in_=wout1_d.ap().rearrange("(k p) c -> p k c", p=128))
        nc.sync.dma_start(out=wout2_sb, in_=wout2_d[:])
        nc.sync.dma_start(out=wout3_sb, in_=wout3_d[:])
        nc.sync.dma_start(out=tix_sb, in_=tix_d[:])
        nc.sync.dma_start(out=wm_sb, in_=wm_d[:])
        nc.sync.dma_start(out=den_sb, in_=den_d[:])
        nc.sync.dma_start(out=rden_sb, in_=rden_d[:])
        nc.sync.dma_start(out=wgq_sb, in_=wgq_d[:])

        nc.vector.memset(ones_sb[:, :], 1.0)
        nc.vector.memset(zix[:, :], 0)

        # ---- cluster hidden states h.T (all batch rows, computed locally) --
        HT_OFF = [0, 128, 256, 384, 512, 640, 768, 896]
        HT_M = [128, 128, 128, 128, 128, 128, 128, 64]

        def emit_h():
          for bc in range(2):
              for htile in range(2):
                  pst = ps.tile([128, 2048], F32, tag="ps", name=f"hps_{bc}_{htile}")
                  for hl in range(4):
                      ht = htile * 4 + hl
                      M = HT_M[ht]
                      if FP8_ON:
                          for kp in range(4):
                              nc.tensor.matmul(
                                  pst[0:M, hl * 512:(hl + 1) * 512],
                                  pj[:, 2 * kp:2 * kp + 2, HT_OFF[ht]:HT_OFF[ht] + M],
                                  xt_sb[:, 2 * kp:2 * kp + 2, bc * 512:(bc + 1) * 512],
                                  start=(kp == 0), stop=(kp == 3), perf_mode=DR)
                      else:
                          for k in range(8):
                              nc.tensor.matmul(
                                  pst[0:M, hl * 512:(hl + 1) * 512],
                                  pj[:, k, HT_OFF[ht]:HT_OFF[ht] + M],
                                  xt_sb[:, k, bc * 512:(bc + 1) * 512],
                                  start=(k == 0), stop=(k == 7))
                  for hl in range(4):
                      ht = htile * 4 + hl
                      src = pst[0:HT_M[ht], hl * 512:(hl + 1) * 512]
                      bsl = slice(bc * 512, (bc + 1) * 512)
                      if ht < 4:
                          nc.scalar.copy(h0_sb[:, ht, bsl], src)
                      elif ht < 6:
                          nc.vector.tensor_copy(h1_sb[:, ht - 4, bsl], src)
                      elif ht == 6:
                          nc.vector.tensor_copy(h2_sb[:, bsl], src)
                      else:
                          nc.vector.tensor_copy(h3_sb[0:64, bsl], src)

        # ---- main loop: logits -> exp (+Z accumulate) -> gather/ln --------
        KW = 2 if FP8_ON else 1

        def lh_head(kp, rt):
            return xt_sb[:, KW * kp:KW * kp + KW, rt * 128:(rt + 1) * 128]

        def lh_c0(kp, rt):
            return h0_sb[:, KW * kp:KW * kp + KW, rt * 128:(rt + 1) * 128]

        def lh_c1(kp, rt):
            return h1_sb[:, KW * kp:KW * kp + KW, rt * 128:(rt + 1) * 128]

        def lh_c2(kp, rt):
            return h2_sb[:, rt * 128:(rt + 1) * 128]

        def lh_c3(kp, rt):
            return h3_sb[0:64, rt * 128:(rt + 1) * 128]

        def rh_head(kp, a, w):
            return whead_sb[:, KW * kp:KW * kp + KW, a:a + w]

        def rh_w0(kp, a, w):
            return wout0_sb[:, KW * kp:KW * kp + KW, a:a + w]

        def rh_w1(kp, a, w):
            return wout1_sb[:, KW * kp:KW * kp + KW, a:a + w]

        def rh_w2(kp, a, w):
            return wout2_sb[:, a:a + w]

        def rh_w3(kp, a, w):
            return wout3_sb[0:64, a:a + w]

        ESC = 1.0 / S_WHEAD                # uniform: all logits land x64
        if FP8_ON:
            GROUPS = [
                (0, 1254, 4, True, lh_head, rh_head),
                (OFF_CL[0], 1250, 2, True, lh_c0, rh_w0),
                (OFF_CL[1], 2500, 1, True, lh_c1, rh_w1),
                (OFF_CL[2], 5000, 1, False, lh_c2, rh_w2),
                (OFF_CL[3], 2500, 1, False, lh_c3, rh_w3),
            ]
        else:
            GROUPS = [
                (0, 1254, 8, False, lh_head, rh_head),
                (OFF_CL[0], 1250, 4, False, lh_c0, rh_w0),
                (OFF_CL[1], 2500, 2, False, lh_c1, rh_w1),
                (OFF_CL[2], 5000, 1, False, lh_c2, rh_w2),
                (OFF_CL[3], 2500, 1, False, lh_c3, rh_w3),
            ]

        # piece emission order for non-hoisted row tiles, chosen so ACT never
        # starves: the PE-heavy head halves hide behind ACT-heavy cluster
        # pieces
        ORDER = [7, 4, 0, 3, 1, 2, 5, 6]

        t8z = sb.tile([128, 8], F32)
        linkexp = sb.tile([128, 32], F32)
        lsum = sb.tile([128, 8], F32)

        HOIST = 3                                 # piece-0s hoisted pre-h

        def emit_piece(rt, pi, expb):
            lo, hi = PB[pi], PB[pi + 1]
            pst = ps.tile([128, hi - lo], F32, tag="ps",
                          name=f"ps_{rt}_{pi}")
            for goff, width, kt, fp8, lh, rh in GROUPS:
                slo, shi = max(goff, lo), min(goff + width, hi)
                if slo >= shi:
                    continue
                subs = []
                a = slo
                while a < shi:
                    w = min(shi - a, 512 - ((a - lo) % 512))
                    subs.append((a, w))
                    a += w
                for kp in range(kt):
                    for a, w in subs:
                        nc.tensor.matmul(
                            pst[:, a - lo:a - lo + w],
                            lh(kp, rt), rh(kp, a - goff, w),
                            start=(kp == 0), stop=(kp == kt - 1),
                            perf_mode=DR if fp8 else None)
            nc.scalar.activation(
                expb[:, lo:hi], pst[:, 0:hi - lo], EXP,
                scale=ESC, accum_out=zs[:, rt, pi:pi + 1])
            if pi == 1:
                # raw link logits out of PSUM (their ln IS the logit)
                nc.vector.tensor_scalar(
                    llinkraw[:, rt, :], pst[:, 1250 - lo:1254 - lo],
                    ESC, None, op0=MULT)

        def emit_zfix(r0, r1):
            # reconstruct per-group Z from piece accumulators + boundary
            # sums for row tiles [r0, r1); Zc0 still needs the link exp sum
            # subtracted once at the end (A1 includes the link cols).
            sl = slice(r0, r1)
            # Zh(partial) = A0 + sv0
            nc.vector.tensor_tensor(pview[:, 0, sl], zs[:, sl, 0],
                                    sv[:, sl, 0], ADD)
            # Zc0 = A1 - sv0 + sv1   (minus lsum at the end)
            nc.vector.tensor_tensor(t8z[:, sl], zs[:, sl, 1], sv[:, sl, 0], SUB)
            nc.vector.tensor_tensor(pview[:, 1, sl], t8z[:, sl],
                                    sv[:, sl, 1], ADD)
            # Zc1 = A2 - sv1 + sv2
            nc.vector.tensor_tensor(t8z[:, sl], zs[:, sl, 2], sv[:, sl, 1], SUB)
            nc.vector.tensor_tensor(pview[:, 2, sl], t8z[:, sl],
                                    sv[:, sl, 2], ADD)
            # Zc2 = A3 - sv2 + A4 + A5 - sv3
            nc.vector.tensor_tensor(t8z[:, sl], zs[:, sl, 3], sv[:, sl, 2], SUB)
            nc.vector.tensor_tensor(t8z[:, sl], t8z[:, sl], zs[:, sl, 4], ADD)
            nc.vector.tensor_tensor(t8z[:, sl], t8z[:, sl], zs[:, sl, 5], ADD)
            nc.vector.tensor_tensor(pview[:, 3, sl], t8z[:, sl],
                                    sv[:, sl, 3], SUB)
            # Zc3 = sv3 + A6 + A7
            nc.vector.tensor_tensor(t8z[:, sl], sv[:, sl, 3], zs[:, sl, 6], ADD)
            nc.vector.tensor_tensor(pview[:, 4, sl], t8z[:, sl],
                                    zs[:, sl, 7], ADD)

        tmp2S = sb.tile([128, 2, S], BF16_DT)

        def emit_numer(r0, r1):
            # ln of gathered exp values + weighted-sum numerator for row
            # tiles [r0, r1) (tensor_tensor_reduce faults on hw — avoid)
            n = r1 - r0
            nc.scalar.activation(
                logv3[:, r0:r1, :].rearrange("p a b -> p (a b)"),
                vg3[:, r0:r1, :].rearrange("p a b -> p (a b)"), LN)
            nc.vector.tensor_tensor(
                tmp2S[:, 0:n, :], logv3[:, r0:r1, :], wm_sb[:, r0:r1, :], MULT)
            nc.vector.tensor_reduce(
                pview[:, 5, r0:r1], tmp2S[:, 0:n, :], AXX, ADD)

        expbs = {}
        for rt in range(HOIST):
            expbs[rt] = big.tile([128, CONCAT_PAD], BF16_DT, tag="big",
                                 name=f"expb_{rt}")
            emit_piece(rt, 0, expbs[rt])

        emit_h()

        for rt in range(RT):
            expb = expbs.get(rt)
            if expb is None:
                expb = big.tile([128, CONCAT_PAD], BF16_DT, tag="big",
                                name=f"expb_{rt}")
            for oi, pi in enumerate(range(1, NPIECE) if rt < HOIST else ORDER):
                emit_piece(rt, pi, expb)
                if rt == RT - 1 and pi in (2, 4, 5):
                    # keep the gpsimd Q7 awake so the final gather doesn't
                    # pay its ~8us wake latency
                    nc.gpsimd.indirect_copy(
                        warmg[:, 0:16], expb[:, PB[pi]:PB[pi] + 16],
                        zix[:, 0:1], True)
                if oi == 5 and rt >= 2 and rt % 2 == 0:
                    emit_numer(rt - 2, rt)
                if oi == 5 and rt == RT - 1:
                    emit_numer(RT - 2, RT - 1)
            # boundary small-side sums on DVE (link cols excluded from sv0)
            for q, (za, zb) in enumerate(SVS):
                nc.vector.tensor_scalar(
                    zscr[:, 0:zb - za],
                    expb[:, za:zb], 1.0, 0.0, op0=MULT, op1=ADD,
                    accum_out=sv[:, rt, q:q + 1])
            if rt == 3 or rt == RT - 1:
                emit_zfix(0 if rt == 3 else 4, rt + 1)
            # gather exp(logit) at this core's targets
            nc.gpsimd.indirect_copy(
                vg3[:, rt, :], expb[:, 0:CONCAT],
                tix_sb[:, rt * SW:(rt + 1) * SW], True)
        emit_numer(RT - 1, RT)

        # link exp sums: computed once, subtracted from the Zc0 partials
        # (A1 includes the replicated link cols; they are added back exactly
        # once into Zh after the cross-core sum)
        nc.scalar.activation(
            linkexp[:, :],
            llinkraw[:, :, :].rearrange("p a b -> p (a b)"), EXP)
        lx3 = linkexp[:, :].rearrange("p (r g) -> p r g", g=4)
        nc.vector.tensor_reduce(lsum[:, :], lx3, AXX, ADD)
        nc.vector.tensor_tensor(pview[:, 1, :], pview[:, 1, :], lsum[:, :], SUB)

        # self slot of the all-gather
        nc.vector.tensor_copy(rgath[:, 0, :], pay[:, :])

        # ---- fire the peer writes, wait for all 7 peers, sum the slots ----
        with tc.tile_critical():
            for k in range(1, 8):
                rdests = [(0, k) if j == k else None for j in range(8)]
                nc.gpsimd.remote_dma_broadcast(
                    rgath[:, k, :], pay[:, :], rsem, lsem,
                    rdests=rdests).then_inc(psem, 1)
            nc.gpsimd.wait_ge(psem, 7)
            nc.gpsimd.bir_kernel_barrier_wait([list(range(NCORES))])
            nc.gpsimd.trigger_dma(count=7)
            nc.vector.wait_ge(rsem, 14)
            for k in range(1, 8):
                nc.vector.tensor_tensor(rsum[:, :],
                                        rgath[:, 0, :] if k == 1 else rsum[:, :],
                                        rgath[:, k, :], ADD)

        # ---- final combine (identical on every core) ----
        zcomb = sb.tile([128, 40], F32)
        lnz = sb.tile([128, 40], F32)
        s8 = sb.tile([128, 8], F32)
        tA = sb.tile([128, 8], F32)
        num8 = sb.tile([128, 8], F32)
        pcol = sb.tile([128, 1], F32)
        llview = llinkraw[:, :, :]

        nc.vector.tensor_tensor(zcomb[:, 0:8], rsq[:, 0, :], lsum[:, :], ADD)
        nc.vector.tensor_copy(zcomb[:, 8:40], rsum[:, 8:40])
        nc.scalar.activation(lnz[:, :], zcomb[:, :], LN)
        llink3 = llview.rearrange("p r g -> p g r")
        for g in range(4):
            nc.vector.tensor_tensor(
                tA[:, :], llink3[:, g, :], lnz[:, 8 + 8 * g:16 + 8 * g], SUB)
            if g == 0:
                nc.vector.tensor_tensor(s8[:, :], tA[:, :], wgq_sb[:, g, :], MULT)
            else:
                nc.vector.tensor_tensor(tA[:, :], tA[:, :], wgq_sb[:, g, :], MULT)
                nc.vector.tensor_tensor(s8[:, :], s8[:, :], tA[:, :], ADD)
        # num = numraw + s8 - den * logZh, scaled by 1/den
        nc.vector.tensor_tensor(tA[:, :], den_sb[:, :], lnz[:, 0:8], MULT)
        nc.vector.tensor_tensor(num8[:, :], rsq[:, 5, :], tA[:, :], SUB)
        nc.vector.tensor_tensor(num8[:, :], num8[:, :], s8[:, :], ADD)
        nc.vector.tensor_tensor(num8[:, :], num8[:, :], rden_sb[:, :], MULT)
        nc.vector.tensor_reduce(pcol[:, :], num8[:, :], AXX, ADD)
        psq = ps.tile([1, 1], F32, tag="ps")
        nc.tensor.matmul(psq[0:1, 0:1], pcol[:, 0:1], ones_sb[:, 0:1],
                         start=True, stop=True)
        nc.scalar.mul(out_sb[:, :], psq[0:1, 0:1], -1.0 / (B + 1e-5))
        nc.sync.dma_start(out=out_d[:], in_=out_sb)
        if DBG:
            nc.sync.dma_start(out=pay_d[:], in_=pay[:, :])
            nc.sync.dma_start(out=rsum_d[:], in_=rsum[:, :])
            nc.sync.dma_start(out=zcomb_d[:], in_=zcomb[:, :])

    nc.compile()
    _CACHE[S] = nc
    return nc


# ----------------------------------------------------------------------------
# host-side sharding / index routing
# ----------------------------------------------------------------------------


def _f8(a, scale):
    return np.clip(np.asarray(a, np.float32) * scale, -239.0, 239.0).astype(F8)


def _shard_inputs(features, head_weight, projs, outs, discard_probs,
                  targets, target_mask):
    """Build the 8 per-core input maps. Returns (in_maps, S)."""
    if FP8_ON:
        xt = _f8(np.ascontiguousarray(features.T), 1.0)
        projt = _f8(np.concatenate([p.T for p in projs], axis=1), S_PROJ)
    else:
        xt = np.ascontiguousarray(features.T).astype(BF16)
        projt = (np.concatenate([p.T for p in projs], axis=1)
                 * S_PROJ).astype(BF16)

    tgt = np.asarray(targets).astype(np.int64).reshape(-1)
    msk = np.asarray(target_mask).astype(bool).reshape(-1)
    bb = np.repeat(np.arange(B, dtype=np.int64), T)

    grp = np.digitize(tgt, GRP_BOUNDS[1:-1])          # 0..4 (0 = shortlist)
    u = tgt - np.asarray(GRP_BOUNDS)[grp]
    shard = np.asarray(GRP_SHARD)[grp]
    core = u // shard
    jcat = u % shard + np.asarray(GRP_OFF)[grp]
    wval = (1.0 - discard_probs[tgt]).astype(np.float32)

    rt = bb >> 7
    gc = (bb >> 4) & 7

    # per-row weight sums: input-only, computed here instead of on-device
    wv = wval * msk
    den_row = np.bincount(bb, weights=wv, minlength=B).astype(np.float32)
    wg_row = np.zeros((B, 4), np.float32)
    for g in range(1, 5):
        selg = grp == g
        wg_row[:, g - 1] = np.bincount(bb[selg], weights=wv[selg],
                                       minlength=B)
    den_in = den_row.reshape(RT, 128).T.copy()            # [p, rt]
    rden_in = (1.0 / np.maximum(den_row, 1e-20)).reshape(RT, 128).T.copy()
    wgq_in = np.ascontiguousarray(
        wg_row.reshape(RT, 128, 4).transpose(1, 2, 0))    # [p, g, rt]

    # padded slots per (core, rt, gc)
    key_all = ((core * RT + rt) * 8 + gc).astype(np.int64)
    valid = msk
    counts = np.bincount(key_all[valid], minlength=NCORES * RT * 8)
    # multiple of 32 so each row-tile's wrapped idx slice stays 4B-aligned
    S = int(counts.max())
    S = ((S + 31) // 32) * 32

    in_maps = []
    for c in range(NCORES):
        sel = valid & (core == c)
        jj = jcat[sel]
        bsel = bb[sel]
        rts = rt[sel]
        gcs = gc[sel]
        ww = wval[sel]
        po = bsel & 15
        key = rts * 8 + gcs
        order = np.argsort(key, kind="stable")
        jj, bsel, rts, gcs, po, ww = (a[order] for a in
                                      (jj, bsel, rts, gcs, po, ww))
        key = key[order]
        # slot within each (rt, gc) bucket
        start_of = np.r_[0, np.flatnonzero(np.diff(key)) + 1]
        bucket_len = np.diff(np.r_[start_of, len(key)])
        slot = np.arange(len(key)) - np.repeat(start_of, bucket_len)

        tix = np.full((128, RT * (S // 16)), PADIDX, np.uint16)
        tix[16 * gcs + slot % 16, rts * (S // 16) + slot // 16] = jj.astype(np.uint16)
        wm = np.zeros((128, RT, S), np.float32)
        wm[16 * gcs + po, rts, slot] = ww
        wm = wm.astype(BF16)

        # head shard + link columns, transposed
        hslice = head_weight[c * SH_SHARD:(c + 1) * SH_SHARD]
        wh_cat = np.concatenate(
            [hslice.T, head_weight[SHORT:SHORT + 4].T], axis=1)
        wheadt = (_f8(wh_cat, S_WHEAD) if FP8_ON
                  else (wh_cat * S_WHEAD).astype(BF16))
        in_maps.append({
            "xt": xt,
            "projt": projt,
            "wheadt": wheadt,
            "wout0t": (_f8(outs[0][c * CL_SHARD[0]:(c + 1) * CL_SHARD[0]].T,
                           S_WOUT) if FP8_ON else
                       (outs[0][c * CL_SHARD[0]:(c + 1) * CL_SHARD[0]].T
                        * S_WOUT).astype(BF16)),
            "wout1t": (_f8(outs[1][c * CL_SHARD[1]:(c + 1) * CL_SHARD[1]].T,
                           S_WOUT) if FP8_ON else
                       (outs[1][c * CL_SHARD[1]:(c + 1) * CL_SHARD[1]].T
                        * S_WOUT).astype(BF16)),
            "wout2t": np.ascontiguousarray(
                outs[2][c * CL_SHARD[2]:(c + 1) * CL_SHARD[2]].T
                * S_W23).astype(BF16),
            "wout3t": np.ascontiguousarray(
                outs[3][c * CL_SHARD[3]:(c + 1) * CL_SHARD[3]].T
                * S_W23).astype(BF16),
            "tgtidx": tix,
            "wm": wm,
            "den": den_in,
            "rden": rden_in,
            "wgq": wgq_in,
        })
    return in_maps, S


def _run(features, head_weight, proj0, out0, proj1, out1, proj2, out2,
         proj3, out3, discard_probs, targets, target_mask,
         trace=False, tmpdir=None):
    features = np.asarray(features, np.float32)
    head_weight = np.asarray(head_weight, np.float32)
    projs = [np.asarray(p, np.float32) for p in (proj0, proj1, proj2, proj3)]
    outs = [np.asarray(o, np.float32) for o in (out0, out1, out2, out3)]
    discard_probs = np.asarray(discard_probs, np.float32)

    in_maps, S = _shard_inputs(features, head_weight, projs, outs,
                               discard_probs, targets, target_mask)
    nc = _build(S)
    res = run_bass_kernel_spmd(nc, in_maps, list(range(NCORES)),
                               trace=trace, tmpdir=tmpdir)
    val = np.asarray(res.results[0]["out"], np.float32).reshape(())
    return val, res


def kernel(**inputs) -> np.ndarray:
    val, _ = _run(**inputs)
    return val


# revision 23
# speedup vs baseline: 1.1227x; 1.1227x over previous
"""Adaptive-softmax loss (nn_AdaptiveLoss) on 8 trn2 NeuronCores.

Strategy: tensor-parallel over the vocab dimension, 8-way. Each core owns
1/8 of the shortlist head columns and 1/8 of each tail cluster's output
rows. Per core:

  - computes cluster hidden states h_g = x @ proj_g.T (replicated, small)
    with fp8 DoubleRow matmuls; h0/h1 requantized to fp8, h2/h3 to bf16,
  - computes its slice of every group's logits: fp8 DoubleRow for the
    K>=256 groups (head/c0/c1), bf16 for the K<=128 clusters (c2/c3);
    weights are prescaled so every group's PSUM logit carries the same
    x64 factor, folded back out via the exp activation scale,
  - exp()s the logits on ACT in 7 PSUM pieces per 128-row tile, with the
    ACT accumulator giving per-piece sums; per-group softmax denominators
    are reconstructed from piece sums plus four narrow boundary sums on
    DVE,
  - gathers exp(logit) at this core's share of the targets straight out
    of SBUF (gpsimd indirect_copy, kept awake by dummy gathers before the
    final row tile) and takes ln in-loop,
  - per-row weight sums (den, W_g) are input-only quantities computed on
    the host and shipped as small tensors - they never ride a collective,
  - the per-row partial stats (5 softmax Zs + weighted-logit numerator,
    [128, 48] fp32) are exchanged with the 7 peers by direct remote SBUF
    DMA writes (XOR-slot all-gather, descriptors generated early and
    triggered at the end), then every core sums the 8 slots and finishes
    the cheap log/normalize arithmetic identically.

The full [B, VOCAB] log-prob matrix is never materialized anywhere, and
no ncfw collective is on the critical path (only the kernel-entry
barrier AllGather, which completes long before the tail needs it).
"""

import sys

sys.path.insert(0, "/opt/trn_rl_repo")

from contextlib import ExitStack

import ml_dtypes
import numpy as np

import concourse.bass as bass  # noqa: F401  (engine types via nc.*)
import concourse.mybir as mybir
import concourse.tile as tile
from concourse import bacc
from concourse.bass_utils import run_bass_kernel_spmd

BF16 = ml_dtypes.bfloat16
F8 = ml_dtypes.float8_e4m3
F32 = mybir.dt.float32
BF16_DT = mybir.dt.bfloat16
F8_DT = mybir.dt.float8e4
U16 = mybir.dt.uint16

NCORES = 8
B, T, D = 1024, 128, 1024
VOCAB, SHORT = 100000, 10000
CL_SIZES = [10000, 20000, 40000, 20000]
CL_D = [512, 256, 128, 64]
SH_SHARD = SHORT // NCORES                      # 1250
CL_SHARD = [s // NCORES for s in CL_SIZES]      # 1250 2500 5000 2500
GRP_BOUNDS = [0, 10000, 20000, 40000, 80000, 100000]
GRP_SHARD = [SH_SHARD] + CL_SHARD

# per-core concatenated logits layout: [head | links(4) | c0 | c1 | c2 | c3]
OFF_HEAD = 0
OFF_LINK = SH_SHARD                              # 1250
OFF_CL = [1254, 2504, 5004, 10004]
GRP_OFF = [OFF_HEAD] + OFF_CL                    # per-group concat offset
CONCAT = OFF_CL[-1] + CL_SHARD[-1]               # 12504
CONCAT_PAD = 12544
# pad slots gather column 0 (always computed, finite); their wm==0 makes
# the contribution vanish.
PADIDX = 0
RT = 8                                           # row tiles of 128

# fp8 scale factors (folded back out via the exp activation scale)
S_WHEAD = 64.0                                   # head weight prescale
S_PROJ = 32.0                                    # proj prescale -> h scale
S_WOUT = 2.0                                     # c0/c1 out-proj prescale
S_W23 = 2.0                                      # c2/c3 out-proj prescale
EXP_TABLE_ID = 6                                 # natural_log_exp_and_others
import os as _os
FP8_ON = not _os.environ.get("ADAK_BF16")

# PSUM piece bounds: head split in two 1024 pieces so its PE-heavy matmuls
# hide behind 2048-wide cluster exps (2-slot PSUM pipeline)
PB = [0, 1024, 2048, 4096, 6144, 8192, 10240, 12288, CONCAT]
NPIECE = 8
# (lo, hi) of the boundary small-side sums, their sv slot = index
SVS = [(1024, 1250), (2048, 2504), (4096, 5004), (10004, 10240)]
# payload stats per row: q = 0 Zh, 1..4 Zc_g, 5 numraw
NSTAT = 6
PAYW = NSTAT * RT                                # 48


# ----------------------------------------------------------------------------
# device kernel builder
# ----------------------------------------------------------------------------

_CACHE: dict[int, object] = {}


def _build(S: int):
    """Build + compile the SPMD kernel for padded slot count S (multiple of 16)."""
    if S in _CACHE:
        return _CACHE[S]
    SW = S // 16

    nc = bacc.Bacc("TRN2", target_bir_lowering=False, debug=False,
                   num_devices=NCORES)

    MMDT = F8_DT if FP8_ON else BF16_DT
    xt_d = nc.dram_tensor("xt", [D, B], MMDT, kind="ExternalInput")
    projt_d = nc.dram_tensor("projt", [D, sum(CL_D)], MMDT, kind="ExternalInput")
    whead_d = nc.dram_tensor("wheadt", [D, 1254], MMDT, kind="ExternalInput")
    wout0_d = nc.dram_tensor("wout0t", [CL_D[0], CL_SHARD[0]], MMDT,
                             kind="ExternalInput")
    wout1_d = nc.dram_tensor("wout1t", [CL_D[1], CL_SHARD[1]], MMDT,
                             kind="ExternalInput")
    wout2_d = nc.dram_tensor("wout2t", [CL_D[2], CL_SHARD[2]], BF16_DT,
                             kind="ExternalInput")
    wout3_d = nc.dram_tensor("wout3t", [CL_D[3], CL_SHARD[3]], BF16_DT,
                             kind="ExternalInput")
    tix_d = nc.dram_tensor("tgtidx", [128, RT * SW], U16, kind="ExternalInput")
    wm_d = nc.dram_tensor("wm", [128, RT, S], BF16_DT, kind="ExternalInput")
    den_d = nc.dram_tensor("den", [128, RT], F32, kind="ExternalInput")
    rden_d = nc.dram_tensor("rden", [128, RT], F32, kind="ExternalInput")
    wgq_d = nc.dram_tensor("wgq", [128, 4, RT], F32, kind="ExternalInput")
    out_d = nc.dram_tensor("out", [1, 1], F32, kind="ExternalOutput")
    DBG = bool(_os.environ.get("ADAK_DBG"))
    if DBG:
        pay_d = nc.dram_tensor("pay_dump", [128, PAYW], F32,
                               kind="ExternalOutput")
        rsum_d = nc.dram_tensor("rsum_dump", [128, PAYW], F32,
                                kind="ExternalOutput")
        zcomb_d = nc.dram_tensor("zcomb_dump", [128, 40], F32,
                                 kind="ExternalOutput")

    EXP = mybir.ActivationFunctionType.Exp
    LN = mybir.ActivationFunctionType.Ln
    ADD = mybir.AluOpType.add
    SUB = mybir.AluOpType.subtract
    MULT = mybir.AluOpType.mult
    AXX = mybir.AxisListType.X
    DR = mybir.MatmulPerfMode.DoubleRow

    with tile.TileContext(nc) as tc, ExitStack() as ctx:
        sb = ctx.enter_context(tc.tile_pool(name="sb", bufs=1))
        big = ctx.enter_context(tc.tile_pool(name="big", bufs=3))
        ps = ctx.enter_context(tc.tile_pool(name="ps", bufs=2, space="PSUM"))

        # combined exp+ln activation table so EXP and LN interleave with a
        # single table load for the whole kernel
        import os
        if not os.environ.get("ADAK_NO_TABLE_PRELOAD"):
            nc.scalar.add_instruction(mybir.InstLoadActFuncSet(
                name=nc.get_next_instruction_name(),
                act_func_set_id=EXP_TABLE_ID, ins=[], outs=[]))

        # ---- persistent SBUF tensors ----
        xt_sb = sb.tile([128, 8, B], MMDT)             # x.T  [d, b] k-tiled
        whead_sb = sb.tile([128, 8, 1254], MMDT)
        wout0_sb = sb.tile([128, 4, CL_SHARD[0]], MMDT)
        wout1_sb = sb.tile([128, 2, CL_SHARD[1]], MMDT)
        wout2_sb = sb.tile([128, CL_SHARD[2]], BF16_DT)
        wout3_sb = sb.tile([64, CL_SHARD[3]], BF16_DT)
        h0_sb = sb.tile([128, 4, B], MMDT)             # h.T (x S_PROJ)
        h1_sb = sb.tile([128, 2, B], MMDT)
        h2_sb = sb.tile([128, B], BF16_DT)
        h3_sb = sb.tile([64, B], BF16_DT)
        tix_sb = sb.tile([128, RT * SW], U16)
        vg3 = sb.tile([128, RT, S], BF16_DT)           # gathered exp(logit)
        wm_sb = sb.tile([128, RT, S], BF16_DT)         # (1-dp)*ownership
        logv3 = sb.tile([128, RT, S], BF16_DT)
        llinkraw = sb.tile([128, RT, 4], F32)          # raw link logits
        zscr = sb.tile([128, 2048], BF16_DT)
        zs = sb.tile([128, RT, NPIECE], F32)  # per-piece exp-sum accumulators
        sv = sb.tile([128, RT, 4], F32)       # boundary small-side sums
        pay1 = sb.tile([128, 40], F32)        # Z stats payload (5 x 8 rt)
        pay2 = sb.tile([128, RT], F32)        # numraw payload
        payh1 = sb.tile([128, 40], BF16_DT)
        payh2 = sb.tile([128, RT], BF16_DT)
        rsb1 = sb.tile([128, 40], BF16_DT)
        rsb2 = sb.tile([128, RT], BF16_DT)
        den_sb = sb.tile([128, RT], F32)
        rden_sb = sb.tile([128, RT], F32)
        wgq_sb = sb.tile([128, 4, RT], F32)
        ones_sb = sb.tile([128, 1], F32)
        warmg = sb.tile([128, 16], BF16_DT)   # gpsimd keep-awake gather dst
        zix = sb.tile([128, 1], U16)
        out_sb = sb.tile([1, 1], F32)

        pview = pay1[:, :].rearrange("p (q r) -> p q r", q=5)
        rsq = rsb1[:, :].rearrange("p (q r) -> p q r", q=5)

        # ---- input DMAs (order matters: compute-critical tensors first;
        # xt/projt interleaved per k-tile so the h matmuls start early) ----
        pj = sb.tile([128, 8, sum(CL_D)], MMDT)
        xt_r = xt_d.ap().rearrange("(k p) b -> p k b", p=128)
        pj_r = projt_d.ap().rearrange("(k p) c -> p k c", p=128)
        wh_r = whead_d.ap().rearrange("(k p) c -> p k c", p=128)
        for k in range(8):
            nc.sync.dma_start(out=xt_sb[:, k, :], in_=xt_r[:, k, :])
            nc.sync.dma_start(out=whead_sb[:, k, :], in_=wh_r[:, k, :])
        for k in range(8):
            nc.sync.dma_start(out=pj[:, k, :], in_=pj_r[:, k, :])
        nc.sync.dma_start(out=wout0_sb,
                          in_=wout0_d.ap().rearrange("(k p) c -> p k c", p=128))
        nc.sync.dma_start(out=wout1_sb,
                          in_=wout1_d.ap().rearrange("(k p) c -> p k c", p=128))
        nc.sync.dma_start(out=wout2_sb, in_=wout2_d[:])
        nc.sync.dma_start(out=wout3_sb, in_=wout3_d[:])
        nc.sync.dma_start(out=tix_sb, in_=tix_d[:])
        nc.sync.dma_start(out=wm_sb, in_=wm_d[:])
        nc.sync.dma_start(out=den_sb, in_=den_d[:])
        nc.sync.dma_start(out=rden_sb, in_=rden_d[:])
        nc.sync.dma_start(out=wgq_sb, in_=wgq_d[:])

        nc.vector.memset(ones_sb[:, :], 1.0)
        nc.vector.memset(zix[:, :], 0)

        # Prewarm the collectives path: dummy AllReduces early in the run
        # absorb the ~60us first-collective ncfw entry barrier and the
        # next-collective setup costs.  Fire-and-forget: nothing reads
        # their results, so no engine queue ever blocks on them.
        dr = ctx.enter_context(tc.tile_pool(name="dr", bufs=1, space="DRAM"))
        warm_src = sb.tile([1, 16], F32)
        nc.vector.memset(warm_src[:, :], 1.0)
        for wi in range(4):
            warm_in = dr.tile([1, 16], F32, name=f"warm_in_{wi}")
            warm_out = dr.tile([1, 16], F32, addr_space="Shared",
                               name=f"warm_out_{wi}")
            nc.sync.dma_start(out=warm_in, in_=warm_src[:, :])
            nc.gpsimd.collective_compute(
                "AllReduce", ADD, replica_groups=[list(range(NCORES))],
                ins=[warm_in.opt()], outs=[warm_out.opt()])

        # ---- cluster hidden states h.T (all batch rows, computed locally) --
        HT_OFF = [0, 128, 256, 384, 512, 640, 768, 896]
        HT_M = [128, 128, 128, 128, 128, 128, 128, 64]

        def emit_h():
          for bc in range(2):
              for htile in range(2):
                  pst = ps.tile([128, 2048], F32, tag="ps", name=f"hps_{bc}_{htile}")
                  for hl in range(4):
                      ht = htile * 4 + hl
                      M = HT_M[ht]
                      if FP8_ON:
                          for kp in range(4):
                              nc.tensor.matmul(
                                  pst[0:M, hl * 512:(hl + 1) * 512],
                                  pj[:, 2 * kp:2 * kp + 2, HT_OFF[ht]:HT_OFF[ht] + M],
                                  xt_sb[:, 2 * kp:2 * kp + 2, bc * 512:(bc + 1) * 512],
                                  start=(kp == 0), stop=(kp == 3), perf_mode=DR)
                      else:
                          for k in range(8):
                              nc.tensor.matmul(
                                  pst[0:M, hl * 512:(hl + 1) * 512],
                                  pj[:, k, HT_OFF[ht]:HT_OFF[ht] + M],
                                  xt_sb[:, k, bc * 512:(bc + 1) * 512],
                                  start=(k == 0), stop=(k == 7))
                  for hl in range(4):
                      ht = htile * 4 + hl
                      src = pst[0:HT_M[ht], hl * 512:(hl + 1) * 512]
                      bsl = slice(bc * 512, (bc + 1) * 512)
                      if ht < 4:
                          nc.scalar.copy(h0_sb[:, ht, bsl], src)
                      elif ht < 6:
                          nc.vector.tensor_copy(h1_sb[:, ht - 4, bsl], src)
                      elif ht == 6:
                          nc.vector.tensor_copy(h2_sb[:, bsl], src)
                      else:
                          nc.vector.tensor_copy(h3_sb[0:64, bsl], src)

        # ---- main loop: logits -> exp (+Z accumulate) -> gather/ln --------
        KW = 2 if FP8_ON else 1

        def lh_head(kp, rt):
            return xt_sb[:, KW * kp:KW * kp + KW, rt * 128:(rt + 1) * 128]

        def lh_c0(kp, rt):
            return h0_sb[:, KW * kp:KW * kp + KW, rt * 128:(rt + 1) * 128]

        def lh_c1(kp, rt):
            return h1_sb[:, KW * kp:KW * kp + KW, rt * 128:(rt + 1) * 128]

        def lh_c2(kp, rt):
            return h2_sb[:, rt * 128:(rt + 1) * 128]

        def lh_c3(kp, rt):
            return h3_sb[0:64, rt * 128:(rt + 1) * 128]

        def rh_head(kp, a, w):
            return whead_sb[:, KW * kp:KW * kp + KW, a:a + w]

        def rh_w0(kp, a, w):
            return wout0_sb[:, KW * kp:KW * kp + KW, a:a + w]

        def rh_w1(kp, a, w):
            return wout1_sb[:, KW * kp:KW * kp + KW, a:a + w]

        def rh_w2(kp, a, w):
            return wout2_sb[:, a:a + w]

        def rh_w3(kp, a, w):
            return wout3_sb[0:64, a:a + w]

        ESC = 1.0 / S_WHEAD                # uniform: all logits land x64
        if FP8_ON:
            GROUPS = [
                (0, 1254, 4, True, lh_head, rh_head),
                (OFF_CL[0], 1250, 2, True, lh_c0, rh_w0),
                (OFF_CL[1], 2500, 1, True, lh_c1, rh_w1),
                (OFF_CL[2], 5000, 1, False, lh_c2, rh_w2),
                (OFF_CL[3], 2500, 1, False, lh_c3, rh_w3),
            ]
        else:
            GROUPS = [
                (0, 1254, 8, False, lh_head, rh_head),
                (OFF_CL[0], 1250, 4, False, lh_c0, rh_w0),
                (OFF_CL[1], 2500, 2, False, lh_c1, rh_w1),
                (OFF_CL[2], 5000, 1, False, lh_c2, rh_w2),
                (OFF_CL[3], 2500, 1, False, lh_c3, rh_w3),
            ]

        # piece emission order for non-hoisted row tiles, chosen so ACT never
        # starves: the PE-heavy head halves hide behind ACT-heavy cluster
        # pieces
        ORDER = [7, 4, 0, 3, 1, 2, 5, 6]

        t8z = sb.tile([128, 8], F32)
        linkexp = sb.tile([128, 32], F32)
        lsum = sb.tile([128, 8], F32)

        HOIST = 3                                 # piece-0s hoisted pre-h

        def emit_piece(rt, pi, expb):
            lo, hi = PB[pi], PB[pi + 1]
            pst = ps.tile([128, hi - lo], F32, tag="ps",
                          name=f"ps_{rt}_{pi}")
            for goff, width, kt, fp8, lh, rh in GROUPS:
                slo, shi = max(goff, lo), min(goff + width, hi)
                if slo >= shi:
                    continue
                subs = []
                a = slo
                while a < shi:
                    w = min(shi - a, 512 - ((a - lo) % 512))
                    subs.append((a, w))
                    a += w
                for kp in range(kt):
                    for a, w in subs:
                        nc.tensor.matmul(
                            pst[:, a - lo:a - lo + w],
                            lh(kp, rt), rh(kp, a - goff, w),
                            start=(kp == 0), stop=(kp == kt - 1),
                            perf_mode=DR if fp8 else None)
            nc.scalar.activation(
                expb[:, lo:hi], pst[:, 0:hi - lo], EXP,
                scale=ESC, accum_out=zs[:, rt, pi:pi + 1])
            if pi == 1:
                # raw link logits out of PSUM (their ln IS the logit)
                nc.vector.tensor_scalar(
                    llinkraw[:, rt, :], pst[:, 1250 - lo:1254 - lo],
                    ESC, None, op0=MULT)

        def emit_zfix(r0, r1):
            # reconstruct per-group Z from piece accumulators + boundary
            # sums for row tiles [r0, r1); Zc0 still needs the link exp sum
            # subtracted once at the end (A1 includes the link cols).
            sl = slice(r0, r1)
            # Zh(partial) = A0 + sv0
            nc.vector.tensor_tensor(pview[:, 0, sl], zs[:, sl, 0],
                                    sv[:, sl, 0], ADD)
            # Zc0 = A1 - sv0 + sv1   (minus lsum at the end)
            nc.vector.tensor_tensor(t8z[:, sl], zs[:, sl, 1], sv[:, sl, 0], SUB)
            nc.vector.tensor_tensor(pview[:, 1, sl], t8z[:, sl],
                                    sv[:, sl, 1], ADD)
            # Zc1 = A2 - sv1 + sv2
            nc.vector.tensor_tensor(t8z[:, sl], zs[:, sl, 2], sv[:, sl, 1], SUB)
            nc.vector.tensor_tensor(pview[:, 2, sl], t8z[:, sl],
                                    sv[:, sl, 2], ADD)
            # Zc2 = A3 - sv2 + A4 + A5 - sv3
            nc.vector.tensor_tensor(t8z[:, sl], zs[:, sl, 3], sv[:, sl, 2], SUB)
            nc.vector.tensor_tensor(t8z[:, sl], t8z[:, sl], zs[:, sl, 4], ADD)
            nc.vector.tensor_tensor(t8z[:, sl], t8z[:, sl], zs[:, sl, 5], ADD)
            nc.vector.tensor_tensor(pview[:, 3, sl], t8z[:, sl],
                                    sv[:, sl, 3], SUB)
            # Zc3 = sv3 + A6 + A7
            nc.vector.tensor_tensor(t8z[:, sl], sv[:, sl, 3], zs[:, sl, 6], ADD)
            nc.vector.tensor_tensor(pview[:, 4, sl], t8z[:, sl],
                                    zs[:, sl, 7], ADD)

        tmp2S = sb.tile([128, 2, S], BF16_DT)

        def emit_numer(r0, r1):
            # ln of gathered exp values + weighted-sum numerator for row
            # tiles [r0, r1) (tensor_tensor_reduce faults on hw — avoid)
            n = r1 - r0
            nc.scalar.activation(
                logv3[:, r0:r1, :].rearrange("p a b -> p (a b)"),
                vg3[:, r0:r1, :].rearrange("p a b -> p (a b)"), LN)
            nc.vector.tensor_tensor(
                tmp2S[:, 0:n, :], logv3[:, r0:r1, :], wm_sb[:, r0:r1, :], MULT)
            nc.vector.tensor_reduce(
                pay2[:, r0:r1], tmp2S[:, 0:n, :], AXX, ADD)

        expbs = {}
        for rt in range(HOIST):
            expbs[rt] = big.tile([128, CONCAT_PAD], BF16_DT, tag="big",
                                 name=f"expb_{rt}")
            emit_piece(rt, 0, expbs[rt])

        emit_h()

        for rt in range(RT):
            expb = expbs.get(rt)
            if expb is None:
                expb = big.tile([128, CONCAT_PAD], BF16_DT, tag="big",
                                name=f"expb_{rt}")
            for oi, pi in enumerate(range(1, NPIECE) if rt < HOIST else ORDER):
                emit_piece(rt, pi, expb)
                if rt == RT - 1 and pi in (2, 4, 5):
                    # keep the gpsimd Q7 awake so the final gather doesn't
                    # pay its ~8us wake latency
                    nc.gpsimd.indirect_copy(
                        warmg[:, 0:16], expb[:, PB[pi]:PB[pi] + 16],
                        zix[:, 0:1], True)
                if oi == 5 and rt >= 2 and rt % 2 == 0:
                    emit_numer(rt - 2, rt)
                if oi == 5 and rt == RT - 1:
                    emit_numer(RT - 2, RT - 1)
            # boundary small-side sums on DVE (link cols excluded from sv0)
            for q, (za, zb) in enumerate(SVS):
                nc.vector.tensor_scalar(
                    zscr[:, 0:zb - za],
                    expb[:, za:zb], 1.0, 0.0, op0=MULT, op1=ADD,
                    accum_out=sv[:, rt, q:q + 1])
            if rt == 3 or rt == RT - 1:
                emit_zfix(0 if rt == 3 else 4, rt + 1)
            # gather exp(logit) at this core's targets
            nc.gpsimd.indirect_copy(
                vg3[:, rt, :], expb[:, 0:CONCAT],
                tix_sb[:, rt * SW:(rt + 1) * SW], True)

        # link exp sums: computed once, subtracted from the Zc0 partials
        # (A1 includes the replicated link cols; they are added back exactly
        # once into Zh after the cross-core sum)
        nc.scalar.activation(
            linkexp[:, :],
            llinkraw[:, :, :].rearrange("p a b -> p (a b)"), EXP)
        lx3 = linkexp[:, :].rearrange("p (r g) -> p r g", g=4)
        nc.vector.tensor_reduce(lsum[:, :], lx3, AXX, ADD)
        nc.vector.tensor_tensor(pview[:, 1, :], pview[:, 1, :], lsum[:, :], SUB)

        # AR#1: the 5 Z stats.  Depends only on the exps/boundary sums, NOT
        # on the target gather, so it fires ~10us before AR#2 and its wire
        # time overlaps the numerator tail.
        cc1_in = dr.tile([128, 40], BF16_DT, name="cc1_in")
        cc1_out = dr.tile([128, 40], BF16_DT, addr_space="Shared",
                          name="cc1_out")
        nc.vector.tensor_copy(payh1[:, :], pay1[:, :])
        nc.sync.dma_start(out=cc1_in, in_=payh1[:, :])
        nc.gpsimd.collective_compute(
            "AllReduce", ADD, replica_groups=[list(range(NCORES))],
            ins=[cc1_in.opt()], outs=[cc1_out.opt()])
        nc.sync.dma_start(out=rsb1, in_=cc1_out)

        emit_numer(RT - 1, RT)

        # AR#2: the per-row weighted-logit numerator (tiny, rides right
        # behind AR#1 on the CC pipeline)
        cc2_in = dr.tile([128, RT], BF16_DT, name="cc2_in")
        cc2_out = dr.tile([128, RT], BF16_DT, addr_space="Shared",
                          name="cc2_out")
        nc.vector.tensor_copy(payh2[:, :], pay2[:, :])
        nc.sync.dma_start(out=cc2_in, in_=payh2[:, :])
        nc.gpsimd.collective_compute(
            "AllReduce", ADD, replica_groups=[list(range(NCORES))],
            ins=[cc2_in.opt()], outs=[cc2_out.opt()])
        nc.sync.dma_start(out=rsb2, in_=cc2_out)

        # ---- final combine (identical on every core) ----
        zcomb = sb.tile([128, 40], F32)
        lnz = sb.tile([128, 40], F32)
        s8 = sb.tile([128, 8], F32)
        tA = sb.tile([128, 8], F32)
        num8 = sb.tile([128, 8], F32)
        pcol = sb.tile([128, 1], F32)
        llview = llinkraw[:, :, :]

        nc.vector.tensor_tensor(zcomb[:, 0:8], rsq[:, 0, :], lsum[:, :], ADD)
        nc.vector.tensor_copy(zcomb[:, 8:40], rsb1[:, 8:40])
        nc.scalar.activation(lnz[:, :], zcomb[:, :], LN)
        llink3 = llview.rearrange("p r g -> p g r")
        for g in range(4):
            nc.vector.tensor_tensor(
                tA[:, :], llink3[:, g, :], lnz[:, 8 + 8 * g:16 + 8 * g], SUB)
            if g == 0:
                nc.vector.tensor_tensor(s8[:, :], tA[:, :], wgq_sb[:, g, :], MULT)
            else:
                nc.vector.tensor_tensor(tA[:, :], tA[:, :], wgq_sb[:, g, :], MULT)
                nc.vector.tensor_tensor(s8[:, :], s8[:, :], tA[:, :], ADD)
        # num = numraw + s8 - den * logZh, scaled by 1/den
        nc.vector.tensor_tensor(tA[:, :], den_sb[:, :], lnz[:, 0:8], MULT)
        nc.vector.tensor_tensor(num8[:, :], rsb2[:, :], tA[:, :], SUB)
        nc.vector.tensor_tensor(num8[:, :], num8[:, :], s8[:, :], ADD)
        nc.vector.tensor_tensor(num8[:, :], num8[:, :], rden_sb[:, :], MULT)
        nc.vector.tensor_reduce(pcol[:, :], num8[:, :], AXX, ADD)
        psq = ps.tile([1, 1], F32, tag="ps")
        nc.tensor.matmul(psq[0:1, 0:1], pcol[:, 0:1], ones_sb[:, 0:1],
                         start=True, stop=True)
        nc.scalar.mul(out_sb[:, :], psq[0:1, 0:1], -1.0 / (B + 1e-5))
        nc.sync.dma_start(out=out_d[:], in_=out_sb)
        if DBG:
            nc.sync.dma_start(out=pay_d[:, 0:40], in_=pay1[:, :])
            nc.sync.dma_start(out=pay_d[:, 40:48], in_=pay2[:, :])
            nc.sync.dma_start(out=rsum_d[:, 0:40], in_=rsb1[:, :])
            nc.sync.dma_start(out=rsum_d[:, 40:48], in_=rsb2[:, :])
            nc.sync.dma_start(out=zcomb_d[:], in_=zcomb[:, :])

    nc.compile()
    _CACHE[S] = nc
    return nc


# ----------------------------------------------------------------------------
# host-side sharding / index routing
# ----------------------------------------------------------------------------


def _f8(a, scale):
    return np.clip(np.asarray(a, np.float32) * scale, -239.0, 239.0).astype(F8)


def _shard_inputs(features, head_weight, projs, outs, discard_probs,
                  targets, target_mask):
    """Build the 8 per-core input maps. Returns (in_maps, S)."""
    if FP8_ON:
        xt = _f8(np.ascontiguousarray(features.T), 1.0)
        projt = _f8(np.concatenate([p.T for p in projs], axis=1), S_PROJ)
    else:
        xt = np.ascontiguousarray(features.T).astype(BF16)
        projt = (np.concatenate([p.T for p in projs], axis=1)
                 * S_PROJ).astype(BF16)

    tgt = np.asarray(targets).astype(np.int64).reshape(-1)
    msk = np.asarray(target_mask).astype(bool).reshape(-1)
    bb = np.repeat(np.arange(B, dtype=np.int64), T)

    grp = np.digitize(tgt, GRP_BOUNDS[1:-1])          # 0..4 (0 = shortlist)
    u = tgt - np.asarray(GRP_BOUNDS)[grp]
    shard = np.asarray(GRP_SHARD)[grp]
    core = u // shard
    jcat = u % shard + np.asarray(GRP_OFF)[grp]
    wval = (1.0 - discard_probs[tgt]).astype(np.float32)

    rt = bb >> 7
    gc = (bb >> 4) & 7

    # per-row weight sums: input-only, computed here instead of on-device
    wv = wval * msk
    den_row = np.bincount(bb, weights=wv, minlength=B).astype(np.float32)
    wg_row = np.zeros((B, 4), np.float32)
    for g in range(1, 5):
        selg = grp == g
        wg_row[:, g - 1] = np.bincount(bb[selg], weights=wv[selg],
                                       minlength=B)
    den_in = den_row.reshape(RT, 128).T.copy()            # [p, rt]
    rden_in = (1.0 / np.maximum(den_row, 1e-20)).reshape(RT, 128).T.copy()
    wgq_in = np.ascontiguousarray(
        wg_row.reshape(RT, 128, 4).transpose(1, 2, 0))    # [p, g, rt]

    # padded slots per (core, rt, gc)
    key_all = ((core * RT + rt) * 8 + gc).astype(np.int64)
    valid = msk
    counts = np.bincount(key_all[valid], minlength=NCORES * RT * 8)
    # multiple of 32 so each row-tile's wrapped idx slice stays 4B-aligned
    S = int(counts.max())
    S = ((S + 31) // 32) * 32

    in_maps = []
    for c in range(NCORES):
        sel = valid & (core == c)
        jj = jcat[sel]
        bsel = bb[sel]
        rts = rt[sel]
        gcs = gc[sel]
        ww = wval[sel]
        po = bsel & 15
        key = rts * 8 + gcs
        order = np.argsort(key, kind="stable")
        jj, bsel, rts, gcs, po, ww = (a[order] for a in
                                      (jj, bsel, rts, gcs, po, ww))
        key = key[order]
        # slot within each (rt, gc) bucket
        start_of = np.r_[0, np.flatnonzero(np.diff(key)) + 1]
        bucket_len = np.diff(np.r_[start_of, len(key)])
        slot = np.arange(len(key)) - np.repeat(start_of, bucket_len)

        tix = np.full((128, RT * (S // 16)), PADIDX, np.uint16)
        tix[16 * gcs + slot % 16, rts * (S // 16) + slot // 16] = jj.astype(np.uint16)
        wm = np.zeros((128, RT, S), np.float32)
        wm[16 * gcs + po, rts, slot] = ww
        wm = wm.astype(BF16)

        # head shard + link columns, transposed
        hslice = head_weight[c * SH_SHARD:(c + 1) * SH_SHARD]
        wh_cat = np.concatenate(
            [hslice.T, head_weight[SHORT:SHORT + 4].T], axis=1)
        wheadt = (_f8(wh_cat, S_WHEAD) if FP8_ON
                  else (wh_cat * S_WHEAD).astype(BF16))
        in_maps.append({
            "xt": xt,
            "projt": projt,
            "wheadt": wheadt,
            "wout0t": (_f8(outs[0][c * CL_SHARD[0]:(c + 1) * CL_SHARD[0]].T,
                           S_WOUT) if FP8_ON else
                       (outs[0][c * CL_SHARD[0]:(c + 1) * CL_SHARD[0]].T
                        * S_WOUT).astype(BF16)),
            "wout1t": (_f8(outs[1][c * CL_SHARD[1]:(c + 1) * CL_SHARD[1]].T,
                           S_WOUT) if FP8_ON else
                       (outs[1][c * CL_SHARD[1]:(c + 1) * CL_SHARD[1]].T
                        * S_WOUT).astype(BF16)),
            "wout2t": np.ascontiguousarray(
                outs[2][c * CL_SHARD[2]:(c + 1) * CL_SHARD[2]].T
                * S_W23).astype(BF16),
            "wout3t": np.ascontiguousarray(
                outs[3][c * CL_SHARD[3]:(c + 1) * CL_SHARD[3]].T
                * S_W23).astype(BF16),
            "tgtidx": tix,
            "wm": wm,
            "den": den_in,
            "rden": rden_in,
            "wgq": wgq_in,
        })
    return in_maps, S


def _run(features, head_weight, proj0, out0, proj1, out1, proj2, out2,
         proj3, out3, discard_probs, targets, target_mask,
         trace=False, tmpdir=None):
    features = np.asarray(features, np.float32)
    head_weight = np.asarray(head_weight, np.float32)
    projs = [np.asarray(p, np.float32) for p in (proj0, proj1, proj2, proj3)]
    outs = [np.asarray(o, np.float32) for o in (out0, out1, out2, out3)]
    discard_probs = np.asarray(discard_probs, np.float32)

    in_maps, S = _shard_inputs(features, head_weight, projs, outs,
                               discard_probs, targets, target_mask)
    nc = _build(S)
    res = run_bass_kernel_spmd(nc, in_maps, list(range(NCORES)),
                               trace=trace, tmpdir=tmpdir)
    val = np.asarray(res.results[0]["out"], np.float32).reshape(())
    return val, res


def kernel(**inputs) -> np.ndarray:
    val, _ = _run(**inputs)
    return val


# revision 59
# speedup vs baseline: 1.3161x; 1.1723x over previous
"""Adaptive-softmax loss (nn_AdaptiveLoss) on 8 trn2 NeuronCores.

Strategy: tensor-parallel over the vocab dimension, 8-way. Each core owns
1/8 of the shortlist head columns and 1/8 of each tail cluster's output
rows. Per core:

  - computes cluster hidden states h_g = x @ proj_g.T (replicated, small)
    with fp8 DoubleRow matmuls; h0/h1 requantized to fp8, h2/h3 to bf16,
  - computes its slice of every group's logits: fp8 DoubleRow for the
    K>=256 groups (head/c0/c1), bf16 for the K<=128 clusters (c2/c3);
    weights are prescaled so every group's PSUM logit carries the same
    x64 factor, folded back out via the exp activation scale,
  - exp()s the logits in 8 PSUM pieces per 128-row tile: six pieces on
    ACT (accumulator gives per-piece sums), the two c3-only pieces via a
    Schraudolph bit-trick exp on DVE (tensor_scalar into int32 +
    bitcast), which moves ~3us/row-tile off the ACT bottleneck at a
    -0.03% softmax-sum bias; per-group Zs are reconstructed from piece
    sums plus four narrow boundary sums on DVE,
  - gathers exp(logit) at this core's share of the targets out of SBUF
    (gpsimd indirect_copy) and takes ln in-loop; the final row tile uses
    two region gathers (the high range [6144:] completes first under the
    piece ORDER) so only a small low-range gather dispatch sits on the
    tail,
  - per-row weight sums (den, W_g) are input-only quantities computed on
    the host and shipped as small tensors - they never ride a collective,
  - one [128, 48] bf16 AllReduce combines the per-row stats (5 softmax
    Zs + weighted-logit numerator); dummy collectives at the start and
    mid-run (rt 4/6) keep the ncfw pipeline warm so the real AllReduce
    doorbell reacts in ~1-3us; every core then finishes the cheap
    log/normalize arithmetic identically and writes the scalar.

The full [B, VOCAB] log-prob matrix is never materialized anywhere.
"""

import sys

sys.path.insert(0, "/opt/trn_rl_repo")

from contextlib import ExitStack

import ml_dtypes
import numpy as np

import concourse.bass as bass  # noqa: F401  (engine types via nc.*)
import concourse.mybir as mybir
import concourse.tile as tile
from concourse import bacc
from concourse.bass_utils import run_bass_kernel_spmd

BF16 = ml_dtypes.bfloat16
F8 = ml_dtypes.float8_e4m3
F32 = mybir.dt.float32
BF16_DT = mybir.dt.bfloat16
F8_DT = mybir.dt.float8e4
U16 = mybir.dt.uint16

NCORES = 8
B, T, D = 1024, 128, 1024
VOCAB, SHORT = 100000, 10000
CL_SIZES = [10000, 20000, 40000, 20000]
CL_D = [512, 256, 128, 64]
SH_SHARD = SHORT // NCORES                      # 1250
CL_SHARD = [s // NCORES for s in CL_SIZES]      # 1250 2500 5000 2500
GRP_BOUNDS = [0, 10000, 20000, 40000, 80000, 100000]
GRP_SHARD = [SH_SHARD] + CL_SHARD

# per-core concatenated logits layout: [head | links(4) | c0 | c1 | c2 | c3]
OFF_HEAD = 0
OFF_LINK = SH_SHARD                              # 1250
OFF_CL = [1254, 2504, 5004, 10004]
GRP_OFF = [OFF_HEAD] + OFF_CL                    # per-group concat offset
CONCAT = OFF_CL[-1] + CL_SHARD[-1]               # 12504
CONCAT_PAD = 12544
# pad slots gather column 0 (always computed, finite); their wm==0 makes
# the contribution vanish.
PADIDX = 0
RT = 8                                           # row tiles of 128

# fp8 scale factors (folded back out via the exp activation scale)
S_WHEAD = 64.0                                   # head weight prescale
S_PROJ = 32.0                                    # proj prescale -> h scale
S_WOUT = 2.0                                     # c0/c1 out-proj prescale
S_W23 = 2.0                                      # c2/c3 out-proj prescale
EXP_TABLE_ID = 6                                 # natural_log_exp_and_others
import os as _os
FP8_ON = not _os.environ.get("ADAK_BF16")

# PSUM piece bounds: head split in two 1024 pieces so its PE-heavy matmuls
# hide behind 2048-wide cluster exps (2-slot PSUM pipeline)
PB = [0, 1024, 2048, 4096, 6144, 8192, 10240, 12288, CONCAT]
NPIECE = 8
# (lo, hi) of the boundary small-side sums, their sv slot = index
SVS = [(1024, 1250), (2048, 2504), (4096, 5004), (10004, 10240)]
# payload stats per row: q = 0 Zh, 1..4 Zc_g, 5 numraw
NSTAT = 6
PAYW = NSTAT * RT                                # 48


# ----------------------------------------------------------------------------
# device kernel builder
# ----------------------------------------------------------------------------

_CACHE: dict[int, object] = {}


def _build(S: int, S7: int):
    """Build + compile the SPMD kernel for padded slot counts S, S7."""
    key = (S, S7, bool(_os.environ.get("ADAK_NOSCHRA")))
    if key in _CACHE:
        return _CACHE[key]
    SW = S // 16
    S7W = S7 // 16

    nc = bacc.Bacc("TRN2", target_bir_lowering=False, debug=False,
                   num_devices=NCORES)

    MMDT = F8_DT if FP8_ON else BF16_DT
    xt_d = nc.dram_tensor("xt", [D, B], MMDT, kind="ExternalInput")
    projt_d = nc.dram_tensor("projt", [D, sum(CL_D)], MMDT, kind="ExternalInput")
    whead_d = nc.dram_tensor("wheadt", [D, 1254], MMDT, kind="ExternalInput")
    wout0_d = nc.dram_tensor("wout0t", [CL_D[0], CL_SHARD[0]], MMDT,
                             kind="ExternalInput")
    wout1_d = nc.dram_tensor("wout1t", [CL_D[1], CL_SHARD[1]], MMDT,
                             kind="ExternalInput")
    wout2_d = nc.dram_tensor("wout2t", [CL_D[2], CL_SHARD[2]], BF16_DT,
                             kind="ExternalInput")
    wout3_d = nc.dram_tensor("wout3t", [CL_D[3], CL_SHARD[3]], BF16_DT,
                             kind="ExternalInput")
    tix_d = nc.dram_tensor("tgtidx", [128, RT * SW], U16, kind="ExternalInput")
    wm_d = nc.dram_tensor("wm", [128, RT, S], BF16_DT, kind="ExternalInput")
    tix7_d = nc.dram_tensor("tgtidx7", [128, 2 * S7W], U16,
                            kind="ExternalInput")
    wm7_d = nc.dram_tensor("wm7", [128, 2, S7], BF16_DT,
                           kind="ExternalInput")
    den_d = nc.dram_tensor("den", [128, RT], F32, kind="ExternalInput")
    rden_d = nc.dram_tensor("rden", [128, RT], F32, kind="ExternalInput")
    wgq_d = nc.dram_tensor("wgq", [128, 4, RT], F32, kind="ExternalInput")
    out_d = nc.dram_tensor("out", [1, 1], F32, kind="ExternalOutput")
    DBG = bool(_os.environ.get("ADAK_DBG"))
    if DBG:
        pay_d = nc.dram_tensor("pay_dump", [128, PAYW], F32,
                               kind="ExternalOutput")
        rsum_d = nc.dram_tensor("rsum_dump", [128, PAYW], F32,
                                kind="ExternalOutput")
        zcomb_d = nc.dram_tensor("zcomb_dump", [128, 40], F32,
                                 kind="ExternalOutput")

    EXP = mybir.ActivationFunctionType.Exp
    LN = mybir.ActivationFunctionType.Ln
    ADD = mybir.AluOpType.add
    SUB = mybir.AluOpType.subtract
    MULT = mybir.AluOpType.mult
    AXX = mybir.AxisListType.X
    DR = mybir.MatmulPerfMode.DoubleRow

    with tile.TileContext(nc) as tc, ExitStack() as ctx:
        sb = ctx.enter_context(tc.tile_pool(name="sb", bufs=1))
        big = ctx.enter_context(tc.tile_pool(name="big", bufs=4))
        ps = ctx.enter_context(tc.tile_pool(name="ps", bufs=2, space="PSUM"))

        # combined exp+ln activation table so EXP and LN interleave with a
        # single table load for the whole kernel
        import os
        if not os.environ.get("ADAK_NO_TABLE_PRELOAD"):
            nc.scalar.add_instruction(mybir.InstLoadActFuncSet(
                name=nc.get_next_instruction_name(),
                act_func_set_id=EXP_TABLE_ID, ins=[], outs=[]))

        # ---- persistent SBUF tensors ----
        xt_sb = sb.tile([128, 8, B], MMDT)             # x.T  [d, b] k-tiled
        whead_sb = sb.tile([128, 8, 1254], MMDT)
        wout0_sb = sb.tile([128, 4, CL_SHARD[0]], MMDT)
        wout1_sb = sb.tile([128, 2, CL_SHARD[1]], MMDT)
        wout2_sb = sb.tile([128, CL_SHARD[2]], BF16_DT)
        wout3_sb = sb.tile([64, CL_SHARD[3]], BF16_DT)
        h0_sb = sb.tile([128, 4, B], MMDT)             # h.T (x S_PROJ)
        h1_sb = sb.tile([128, 2, B], MMDT)
        h2_sb = sb.tile([128, B], BF16_DT)
        h3_sb = sb.tile([64, B], BF16_DT)
        tix_sb = sb.tile([128, RT * SW], U16)
        vg3 = sb.tile([128, RT, S], BF16_DT)           # gathered exp(logit)
        wm_sb = sb.tile([128, RT, S], BF16_DT)         # (1-dp)*ownership
        logv3 = sb.tile([128, RT, S], BF16_DT)
        tix7_sb = sb.tile([128, 2 * S7W], U16)    # rt7: two region planes
        wm7_sb = sb.tile([128, 2, S7], BF16_DT)
        vg7 = sb.tile([128, 2, S7], BF16_DT)
        logv7 = sb.tile([128, 2, S7], BF16_DT)
        tmp7 = sb.tile([128, 2, S7], BF16_DT)
        llinkraw = sb.tile([128, RT, 4], F32)          # raw link logits
        zscr = sb.tile([128, 2048], BF16_DT)
        zs = sb.tile([128, RT, NPIECE], F32)  # per-piece exp-sum accumulators
        sv = sb.tile([128, RT, 4], F32)       # boundary small-side sums
        pay1 = sb.tile([128, 40], F32)        # Z stats payload (5 x 8 rt)
        pay2 = sb.tile([128, RT], F32)        # numraw payload
        payh = sb.tile([128, 48], BF16_DT)
        rsb = sb.tile([128, 48], BF16_DT)
        den_sb = sb.tile([128, RT], F32)
        rden_sb = sb.tile([128, RT], F32)
        wgq_sb = sb.tile([128, 4, RT], F32)
        ones_sb = sb.tile([128, 1], F32)
        out_sb = sb.tile([1, 1], F32)

        pview = pay1[:, :].rearrange("p (q r) -> p q r", q=5)
        rsq = rsb[:, 0:40].rearrange("p (q r) -> p q r", q=5)

        # ---- input DMAs (order matters: compute-critical tensors first;
        # xt/projt interleaved per k-tile so the h matmuls start early) ----
        pj = sb.tile([128, 8, sum(CL_D)], MMDT)
        xt_r = xt_d.ap().rearrange("(k p) b -> p k b", p=128)
        pj_r = projt_d.ap().rearrange("(k p) c -> p k c", p=128)
        wh_r = whead_d.ap().rearrange("(k p) c -> p k c", p=128)
        for k in range(8):
            nc.sync.dma_start(out=xt_sb[:, k, :], in_=xt_r[:, k, :])
            nc.sync.dma_start(out=whead_sb[:, k, :], in_=wh_r[:, k, :])
        for k in range(8):
            nc.sync.dma_start(out=pj[:, k, :], in_=pj_r[:, k, :])
        nc.sync.dma_start(out=wout0_sb,
                          in_=wout0_d.ap().rearrange("(k p) c -> p k c", p=128))
        nc.sync.dma_start(out=wout1_sb,
                          in_=wout1_d.ap().rearrange("(k p) c -> p k c", p=128))
        nc.sync.dma_start(out=wout2_sb, in_=wout2_d[:])
        nc.sync.dma_start(out=wout3_sb, in_=wout3_d[:])
        nc.sync.dma_start(out=tix_sb, in_=tix_d[:])
        nc.sync.dma_start(out=wm_sb, in_=wm_d[:])
        nc.sync.dma_start(out=tix7_sb, in_=tix7_d[:])
        nc.sync.dma_start(out=wm7_sb, in_=wm7_d[:])
        nc.sync.dma_start(out=den_sb, in_=den_d[:])
        nc.sync.dma_start(out=rden_sb, in_=rden_d[:])
        nc.sync.dma_start(out=wgq_sb, in_=wgq_d[:])

        nc.vector.memset(ones_sb[:, :], 1.0)

        # Prewarm the collectives path: dummy AllReduces early in the run
        # absorb the ~60us first-collective ncfw entry barrier and the
        # next-collective setup costs.  Fire-and-forget: nothing reads
        # their results, so no engine queue ever blocks on them.
        dr = ctx.enter_context(tc.tile_pool(name="dr", bufs=1, space="DRAM"))
        warm_src = sb.tile([1, 16], F32)
        nc.vector.memset(warm_src[:, :], 1.0)
        for wi in range(4):
            warm_in = dr.tile([1, 16], F32, name=f"warm_in_{wi}")
            warm_out = dr.tile([1, 16], F32, addr_space="Shared",
                               name=f"warm_out_{wi}")
            nc.sync.dma_start(out=warm_in, in_=warm_src[:, :])
            nc.gpsimd.collective_compute(
                "AllReduce", ADD, replica_groups=[list(range(NCORES))],
                ins=[warm_in.opt()], outs=[warm_out.opt()])

        # ---- cluster hidden states h.T (all batch rows, computed locally) --
        HT_OFF = [0, 128, 256, 384, 512, 640, 768, 896]
        HT_M = [128, 128, 128, 128, 128, 128, 128, 64]

        def emit_h():
          for bc in range(2):
              for htile in range(2):
                  pst = ps.tile([128, 2048], F32, tag="ps", name=f"hps_{bc}_{htile}")
                  for hl in range(4):
                      ht = htile * 4 + hl
                      M = HT_M[ht]
                      if FP8_ON:
                          for kp in range(4):
                              nc.tensor.matmul(
                                  pst[0:M, hl * 512:(hl + 1) * 512],
                                  pj[:, 2 * kp:2 * kp + 2, HT_OFF[ht]:HT_OFF[ht] + M],
                                  xt_sb[:, 2 * kp:2 * kp + 2, bc * 512:(bc + 1) * 512],
                                  start=(kp == 0), stop=(kp == 3), perf_mode=DR)
                      else:
                          for k in range(8):
                              nc.tensor.matmul(
                                  pst[0:M, hl * 512:(hl + 1) * 512],
                                  pj[:, k, HT_OFF[ht]:HT_OFF[ht] + M],
                                  xt_sb[:, k, bc * 512:(bc + 1) * 512],
                                  start=(k == 0), stop=(k == 7))
                  for hl in range(4):
                      ht = htile * 4 + hl
                      src = pst[0:HT_M[ht], hl * 512:(hl + 1) * 512]
                      bsl = slice(bc * 512, (bc + 1) * 512)
                      if ht < 4:
                          nc.scalar.copy(h0_sb[:, ht, bsl], src)
                      elif ht < 6:
                          nc.vector.tensor_copy(h1_sb[:, ht - 4, bsl], src)
                      elif ht == 6:
                          nc.vector.tensor_copy(h2_sb[:, bsl], src)
                      else:
                          nc.vector.tensor_copy(h3_sb[0:64, bsl], src)

        # ---- main loop: logits -> exp (+Z accumulate) -> gather/ln --------
        KW = 2 if FP8_ON else 1

        def lh_head(kp, rt):
            return xt_sb[:, KW * kp:KW * kp + KW, rt * 128:(rt + 1) * 128]

        def lh_c0(kp, rt):
            return h0_sb[:, KW * kp:KW * kp + KW, rt * 128:(rt + 1) * 128]

        def lh_c1(kp, rt):
            return h1_sb[:, KW * kp:KW * kp + KW, rt * 128:(rt + 1) * 128]

        def lh_c2(kp, rt):
            return h2_sb[:, rt * 128:(rt + 1) * 128]

        def lh_c3(kp, rt):
            return h3_sb[0:64, rt * 128:(rt + 1) * 128]

        def rh_head(kp, a, w):
            return whead_sb[:, KW * kp:KW * kp + KW, a:a + w]

        def rh_w0(kp, a, w):
            return wout0_sb[:, KW * kp:KW * kp + KW, a:a + w]

        def rh_w1(kp, a, w):
            return wout1_sb[:, KW * kp:KW * kp + KW, a:a + w]

        def rh_w2(kp, a, w):
            return wout2_sb[:, a:a + w]

        def rh_w3(kp, a, w):
            return wout3_sb[0:64, a:a + w]

        ESC = 1.0 / S_WHEAD                # uniform: all logits land x64
        if FP8_ON:
            GROUPS = [
                (0, 1254, 4, True, lh_head, rh_head),
                (OFF_CL[0], 1250, 2, True, lh_c0, rh_w0),
                (OFF_CL[1], 2500, 1, True, lh_c1, rh_w1),
                (OFF_CL[2], 5000, 1, False, lh_c2, rh_w2),
                (OFF_CL[3], 2500, 1, False, lh_c3, rh_w3),
            ]
        else:
            GROUPS = [
                (0, 1254, 8, False, lh_head, rh_head),
                (OFF_CL[0], 1250, 4, False, lh_c0, rh_w0),
                (OFF_CL[1], 2500, 2, False, lh_c1, rh_w1),
                (OFF_CL[2], 5000, 1, False, lh_c2, rh_w2),
                (OFF_CL[3], 2500, 1, False, lh_c3, rh_w3),
            ]

        # piece emission order for non-hoisted row tiles, chosen so ACT never
        # starves (the PE-heavy head halves hide behind ACT-heavy cluster
        # pieces), the two DVE-exp pieces (6, 7) sit apart, and the high
        # column range [6144, 12504) = pieces {4,5,6,7} completes first so
        # the final row tile's high-region gather fires early
        ORDER = [7, 4, 5, 6, 0, 3, 1, 2]

        t8z = sb.tile([128, 8], F32)
        linkexp = sb.tile([128, 32], F32)
        lsum = sb.tile([128, 8], F32)
        i32scr = sb.tile([128, 2048], mybir.dt.int32)
        SCHRA_A = ESC * (2.0 ** 23 / float(np.log(2.0)))
        SCHRA_B = float(127 * 2 ** 23 - 486411)

        HOIST = 3                                 # piece-0s hoisted pre-h

        def emit_piece(rt, pi, expb):
            lo, hi = PB[pi], PB[pi + 1]
            pst = ps.tile([128, hi - lo], F32, tag="ps",
                          name=f"ps_{rt}_{pi}")
            for goff, width, kt, fp8, lh, rh in GROUPS:
                slo, shi = max(goff, lo), min(goff + width, hi)
                if slo >= shi:
                    continue
                subs = []
                a = slo
                while a < shi:
                    w = min(shi - a, 512 - ((a - lo) % 512))
                    subs.append((a, w))
                    a += w
                for kp in range(kt):
                    for a, w in subs:
                        nc.tensor.matmul(
                            pst[:, a - lo:a - lo + w],
                            lh(kp, rt), rh(kp, a - goff, w),
                            start=(kp == 0), stop=(kp == kt - 1),
                            perf_mode=DR if fp8 else None)
            if pi >= 6 and not _os.environ.get('ADAK_NOSCHRA'):
                # Schraudolph bit-trick exp on DVE for the two c3-only
                # pieces: exp(l) ~ bitcast_f32(round(A*psum + B)).  Offloads
                # ~3us/rt from the ACT bottleneck; the +-4% element error
                # only touches the c3 softmax sum (bias -0.03%) and the
                # gathered c3 target values.
                W = hi - lo
                nc.vector.tensor_scalar(
                    i32scr[:, 0:W], pst[:, 0:W], SCHRA_A, SCHRA_B,
                    op0=MULT, op1=ADD)
                nc.vector.tensor_scalar(
                    expb[:, lo:hi], i32scr[:, 0:W].bitcast(F32), 1.0, 0.0,
                    op0=MULT, op1=ADD, accum_out=zs[:, rt, pi:pi + 1])
            else:
                nc.scalar.activation(
                    expb[:, lo:hi], pst[:, 0:hi - lo], EXP,
                    scale=ESC, accum_out=zs[:, rt, pi:pi + 1])
            if pi == 1:
                # raw link logits out of PSUM (their ln IS the logit)
                nc.vector.tensor_scalar(
                    llinkraw[:, rt, :], pst[:, 1250 - lo:1254 - lo],
                    ESC, None, op0=MULT)

        def emit_zfix(r0, r1):
            # reconstruct per-group Z from piece accumulators + boundary
            # sums for row tiles [r0, r1); Zc0 still needs the link exp sum
            # subtracted once at the end (A1 includes the link cols).
            sl = slice(r0, r1)
            # Zh(partial) = A0 + sv0
            nc.vector.tensor_tensor(pview[:, 0, sl], zs[:, sl, 0],
                                    sv[:, sl, 0], ADD)
            # Zc0 = A1 - sv0 + sv1   (minus lsum at the end)
            nc.vector.tensor_tensor(t8z[:, sl], zs[:, sl, 1], sv[:, sl, 0], SUB)
            nc.vector.tensor_tensor(pview[:, 1, sl], t8z[:, sl],
                                    sv[:, sl, 1], ADD)
            # Zc1 = A2 - sv1 + sv2
            nc.vector.tensor_tensor(t8z[:, sl], zs[:, sl, 2], sv[:, sl, 1], SUB)
            nc.vector.tensor_tensor(pview[:, 2, sl], t8z[:, sl],
                                    sv[:, sl, 2], ADD)
            # Zc2 = A3 - sv2 + A4 + A5 - sv3
            nc.vector.tensor_tensor(t8z[:, sl], zs[:, sl, 3], sv[:, sl, 2], SUB)
            nc.vector.tensor_tensor(t8z[:, sl], t8z[:, sl], zs[:, sl, 4], ADD)
            nc.vector.tensor_tensor(t8z[:, sl], t8z[:, sl], zs[:, sl, 5], ADD)
            nc.vector.tensor_tensor(pview[:, 3, sl], t8z[:, sl],
                                    sv[:, sl, 3], SUB)
            # Zc3 = sv3 + A6 + A7
            nc.vector.tensor_tensor(t8z[:, sl], sv[:, sl, 3], zs[:, sl, 6], ADD)
            nc.vector.tensor_tensor(pview[:, 4, sl], t8z[:, sl],
                                    zs[:, sl, 7], ADD)

        tmp2S = sb.tile([128, 2, S], BF16_DT)

        def emit_numer(r0, r1):
            # ln of gathered exp values + weighted-sum numerator for row
            # tiles [r0, r1) (tensor_tensor_reduce faults on hw — avoid)
            n = r1 - r0
            nc.scalar.activation(
                logv3[:, r0:r1, :].rearrange("p a b -> p (a b)"),
                vg3[:, r0:r1, :].rearrange("p a b -> p (a b)"), LN)
            nc.vector.tensor_tensor(
                tmp2S[:, 0:n, :], logv3[:, r0:r1, :], wm_sb[:, r0:r1, :], MULT)
            nc.vector.tensor_reduce(
                pay2[:, r0:r1], tmp2S[:, 0:n, :], AXX, ADD)

        expbs = {}
        for rt in range(HOIST):
            expbs[rt] = big.tile([128, CONCAT_PAD], BF16_DT, tag="big",
                                 name=f"expb_{rt}")
            emit_piece(rt, 0, expbs[rt])

        emit_h()

        for rt in range(RT):
            expb = expbs.get(rt)
            if expb is None:
                expb = big.tile([128, CONCAT_PAD], BF16_DT, tag="big",
                                name=f"expb_{rt}")
            for oi, pi in enumerate(range(1, NPIECE) if rt < HOIST else ORDER):
                emit_piece(rt, pi, expb)
                if oi == 5 and rt >= 2 and rt % 2 == 0:
                    emit_numer(rt - 2, rt)
                if oi == 5 and rt == RT - 1:
                    emit_numer(RT - 2, RT - 1)
            # boundary small-side sums on DVE (link cols excluded from sv0)
            for q, (za, zb) in enumerate(SVS):
                nc.vector.tensor_scalar(
                    zscr[:, 0:zb - za],
                    expb[:, za:zb], 1.0, 0.0, op0=MULT, op1=ADD,
                    accum_out=sv[:, rt, q:q + 1])
            if rt == 3 or rt == RT - 1:
                emit_zfix(0 if rt == 3 else 4, rt + 1)
            # gather exp(logit) at this core's targets; the final row tile
            # is split into two region gathers: the high range [6144, 12504)
            # completes early under ORDER, so only the low-range gather's
            # dispatch latency is exposed at the end
            if rt < RT - 1:
                nc.gpsimd.indirect_copy(
                    vg3[:, rt, :], expb[:, 0:CONCAT],
                    tix_sb[:, rt * SW:(rt + 1) * SW], True)
            else:
                nc.gpsimd.indirect_copy(
                    vg7[:, 0, :], expb[:, 6144:CONCAT],
                    tix7_sb[:, 0:S7W], True)
                nc.gpsimd.indirect_copy(
                    vg7[:, 1, :], expb[:, 0:6144],
                    tix7_sb[:, S7W:2 * S7W], True)
            # keep the ncfw collectives pipeline warm mid-run so the real
            # AllReduce doorbell reacts in ~1us instead of ~7us
            if rt in (4, 6):
                warm_in = dr.tile([1, 16], BF16_DT, name=f"warm_mid_{rt}")
                warm_out = dr.tile([1, 16], BF16_DT, addr_space="Shared",
                                   name=f"warm_mid_out_{rt}")
                nc.sync.dma_start(out=warm_in, in_=zscr[0:1, 0:16])
                nc.gpsimd.collective_compute(
                    "AllReduce", ADD, replica_groups=[list(range(NCORES))],
                    ins=[warm_in.opt()], outs=[warm_out.opt()])

        # link exp sums: computed once, subtracted from the Zc0 partials
        # (A1 includes the replicated link cols; they are added back exactly
        # once into Zh after the cross-core sum)
        nc.scalar.activation(
            linkexp[:, :],
            llinkraw[:, :, :].rearrange("p a b -> p (a b)"), EXP)
        lx3 = linkexp[:, :].rearrange("p (r g) -> p r g", g=4)
        nc.vector.tensor_reduce(lsum[:, :], lx3, AXX, ADD)
        nc.vector.tensor_tensor(pview[:, 1, :], pview[:, 1, :], lsum[:, :], SUB)

        # numerator for the final row tile from the per-piece gather planes
        nc.scalar.activation(
            logv7[:, :, :].rearrange("p a b -> p (a b)"),
            vg7[:, :, :].rearrange("p a b -> p (a b)"), LN)
        nc.vector.tensor_tensor(
            tmp7[:, :, :], logv7[:, :, :], wm7_sb[:, :, :], MULT)
        nc.vector.tensor_reduce(
            pay2[:, RT - 1:RT], tmp7[:, :, :].rearrange("p a b -> p (a b)"),
            AXX, ADD)

        # one AllReduce with all 6 stats; at ~14us of fixed ncfw cost per
        # collective, one late AR beats an early Z-AR plus a serialized
        # second one
        cc_in = dr.tile([128, 48], BF16_DT, name="cc_in")
        cc_out = dr.tile([128, 48], BF16_DT, addr_space="Shared",
                         name="cc_out")
        nc.vector.tensor_copy(payh[:, 0:40], pay1[:, :])
        nc.vector.tensor_copy(payh[:, 40:48], pay2[:, :])
        nc.sync.dma_start(out=cc_in, in_=payh[:, :])
        nc.gpsimd.collective_compute(
            "AllReduce", ADD, replica_groups=[list(range(NCORES))],
            ins=[cc_in.opt()], outs=[cc_out.opt()])
        nc.sync.dma_start(out=rsb, in_=cc_out)

        # ---- final combine (identical on every core) ----
        zcomb = sb.tile([128, 40], F32)
        lnz = sb.tile([128, 40], F32)
        s8 = sb.tile([128, 8], F32)
        tA = sb.tile([128, 8], F32)
        num8 = sb.tile([128, 8], F32)
        pcol = sb.tile([128, 1], F32)
        llview = llinkraw[:, :, :]

        nc.vector.tensor_tensor(zcomb[:, 0:8], rsq[:, 0, :], lsum[:, :], ADD)
        nc.vector.tensor_copy(zcomb[:, 8:40], rsb[:, 8:40])
        nc.scalar.activation(lnz[:, :], zcomb[:, :], LN)
        llink3 = llview.rearrange("p r g -> p g r")
        t32 = sb.tile([128, 4, 8], F32)
        nc.vector.tensor_tensor(
            t32[:, :, :].rearrange("p a b -> p (a b)"),
            llink3, lnz[:, 8:40], SUB)
        nc.vector.tensor_tensor(
            t32[:, :, :].rearrange("p a b -> p (a b)"),
            t32[:, :, :].rearrange("p a b -> p (a b)"),
            wgq_sb[:, :, :].rearrange("p a b -> p (a b)"), MULT)
        nc.vector.tensor_tensor(tA[:, :], t32[:, 0, :], t32[:, 1, :], ADD)
        nc.vector.tensor_tensor(s8[:, :], t32[:, 2, :], t32[:, 3, :], ADD)
        nc.vector.tensor_tensor(s8[:, :], s8[:, :], tA[:, :], ADD)
        # num = numraw + s8 - den * logZh, scaled by 1/den
        nc.vector.tensor_tensor(tA[:, :], den_sb[:, :], lnz[:, 0:8], MULT)
        nc.vector.tensor_tensor(num8[:, :], rsb[:, 40:48], tA[:, :], SUB)
        nc.vector.tensor_tensor(num8[:, :], num8[:, :], s8[:, :], ADD)
        nc.vector.tensor_tensor(num8[:, :], num8[:, :], rden_sb[:, :], MULT)
        nc.vector.tensor_reduce(pcol[:, :], num8[:, :], AXX, ADD)
        psq = ps.tile([1, 1], F32, tag="ps")
        nc.tensor.matmul(psq[0:1, 0:1], pcol[:, 0:1], ones_sb[:, 0:1],
                         start=True, stop=True)
        nc.scalar.mul(out_sb[:, :], psq[0:1, 0:1], -1.0 / (B + 1e-5))
        nc.sync.dma_start(out=out_d[:], in_=out_sb)
        if DBG:
            nc.sync.dma_start(out=pay_d[:, 0:40], in_=pay1[:, :])
            nc.sync.dma_start(out=pay_d[:, 40:48], in_=pay2[:, :])
            nc.sync.dma_start(out=rsum_d[:], in_=rsb[:, :])
            nc.sync.dma_start(out=zcomb_d[:], in_=zcomb[:, :])

    nc.compile()
    _CACHE[key] = nc
    return nc


# ----------------------------------------------------------------------------
# host-side sharding / index routing
# ----------------------------------------------------------------------------


def _f8(a, scale):
    return np.clip(np.asarray(a, np.float32) * scale, -239.0, 239.0).astype(F8)


def _shard_inputs(features, head_weight, projs, outs, discard_probs,
                  targets, target_mask):
    """Build the 8 per-core input maps. Returns (in_maps, S)."""
    if FP8_ON:
        xt = _f8(np.ascontiguousarray(features.T), 1.0)
        projt = _f8(np.concatenate([p.T for p in projs], axis=1), S_PROJ)
    else:
        xt = np.ascontiguousarray(features.T).astype(BF16)
        projt = (np.concatenate([p.T for p in projs], axis=1)
                 * S_PROJ).astype(BF16)

    tgt = np.asarray(targets).astype(np.int64).reshape(-1)
    msk = np.asarray(target_mask).astype(bool).reshape(-1)
    bb = np.repeat(np.arange(B, dtype=np.int64), T)

    grp = np.digitize(tgt, GRP_BOUNDS[1:-1])          # 0..4 (0 = shortlist)
    u = tgt - np.asarray(GRP_BOUNDS)[grp]
    shard = np.asarray(GRP_SHARD)[grp]
    core = u // shard
    jcat = u % shard + np.asarray(GRP_OFF)[grp]
    wval = (1.0 - discard_probs[tgt]).astype(np.float32)

    rt = bb >> 7
    gc = (bb >> 4) & 7

    # per-row weight sums: input-only, computed here instead of on-device
    wv = wval * msk
    den_row = np.bincount(bb, weights=wv, minlength=B).astype(np.float32)
    wg_row = np.zeros((B, 4), np.float32)
    for g in range(1, 5):
        selg = grp == g
        wg_row[:, g - 1] = np.bincount(bb[selg], weights=wv[selg],
                                       minlength=B)
    den_in = den_row.reshape(RT, 128).T.copy()            # [p, rt]
    rden_in = (1.0 / np.maximum(den_row, 1e-20)).reshape(RT, 128).T.copy()
    wgq_in = np.ascontiguousarray(
        wg_row.reshape(RT, 128, 4).transpose(1, 2, 0))    # [p, g, rt]

    # piece id of each target (for the final row tile's per-piece gathers)
    pidx = np.searchsorted(np.asarray(PB[1:]), jcat, side="right")

    # padded slots per (core, rt, gc) for rts 0..6, and per (core, piece, gc)
    # for rt 7
    valid = msk
    v_lo = valid & (rt < RT - 1)
    v_hi = valid & (rt == RT - 1)
    key_all = ((core * RT + rt) * 8 + gc).astype(np.int64)
    counts = np.bincount(key_all[v_lo], minlength=NCORES * RT * 8)
    reg = (jcat < 6144).astype(np.int64)
    key7_all = ((core * 2 + reg) * 8 + gc).astype(np.int64)
    counts7 = np.bincount(key7_all[v_hi], minlength=NCORES * 2 * 8)
    # multiple of 32 so each wrapped idx slice stays 4B-aligned
    S = ((int(counts.max()) + 31) // 32) * 32
    S7 = ((int(counts7.max()) + 31) // 32) * 32

    in_maps = []
    for c in range(NCORES):
        sel = v_lo & (core == c)
        jj = jcat[sel]
        bsel = bb[sel]
        rts = rt[sel]
        gcs = gc[sel]
        ww = wval[sel]
        po = bsel & 15
        key = rts * 8 + gcs
        order = np.argsort(key, kind="stable")
        jj, bsel, rts, gcs, po, ww = (a[order] for a in
                                      (jj, bsel, rts, gcs, po, ww))
        key = key[order]
        # slot within each (rt, gc) bucket
        start_of = np.r_[0, np.flatnonzero(np.diff(key)) + 1]
        bucket_len = np.diff(np.r_[start_of, len(key)])
        slot = np.arange(len(key)) - np.repeat(start_of, bucket_len)

        tix = np.full((128, RT * (S // 16)), PADIDX, np.uint16)
        tix[16 * gcs + slot % 16, rts * (S // 16) + slot // 16] = jj.astype(np.uint16)
        wm = np.zeros((128, RT, S), np.float32)
        wm[16 * gcs + po, rts, slot] = ww
        wm = wm.astype(BF16)

        # final row tile: two region planes with region-relative indices
        sel7 = v_hi & (core == c)
        jj7 = jcat[sel7]
        r7 = (jj7 < 6144).astype(np.int64)      # 0 = high region, 1 = low
        gc7 = gc[sel7]
        po7 = bb[sel7] & 15
        ww7 = wval[sel7]
        key7 = r7 * 8 + gc7
        o7 = np.argsort(key7, kind="stable")
        jj7, r7, gc7, po7, ww7 = (a[o7] for a in (jj7, r7, gc7, po7, ww7))
        key7 = key7[o7]
        so7 = np.r_[0, np.flatnonzero(np.diff(key7)) + 1]
        bl7 = np.diff(np.r_[so7, len(key7)])
        slot7 = np.arange(len(key7)) - np.repeat(so7, bl7)
        rel7 = jj7 - np.where(r7 == 0, 6144, 0)
        tix7 = np.full((128, 2 * (S7 // 16)), PADIDX, np.uint16)
        tix7[16 * gc7 + slot7 % 16,
             r7 * (S7 // 16) + slot7 // 16] = rel7.astype(np.uint16)
        wm7 = np.zeros((128, 2, S7), np.float32)
        wm7[16 * gc7 + po7, r7, slot7] = ww7
        wm7 = wm7.astype(BF16)

        # head shard + link columns, transposed
        hslice = head_weight[c * SH_SHARD:(c + 1) * SH_SHARD]
        wh_cat = np.concatenate(
            [hslice.T, head_weight[SHORT:SHORT + 4].T], axis=1)
        wheadt = (_f8(wh_cat, S_WHEAD) if FP8_ON
                  else (wh_cat * S_WHEAD).astype(BF16))
        in_maps.append({
            "xt": xt,
            "projt": projt,
            "wheadt": wheadt,
            "wout0t": (_f8(outs[0][c * CL_SHARD[0]:(c + 1) * CL_SHARD[0]].T,
                           S_WOUT) if FP8_ON else
                       (outs[0][c * CL_SHARD[0]:(c + 1) * CL_SHARD[0]].T
                        * S_WOUT).astype(BF16)),
            "wout1t": (_f8(outs[1][c * CL_SHARD[1]:(c + 1) * CL_SHARD[1]].T,
                           S_WOUT) if FP8_ON else
                       (outs[1][c * CL_SHARD[1]:(c + 1) * CL_SHARD[1]].T
                        * S_WOUT).astype(BF16)),
            "wout2t": np.ascontiguousarray(
                outs[2][c * CL_SHARD[2]:(c + 1) * CL_SHARD[2]].T
                * S_W23).astype(BF16),
            "wout3t": np.ascontiguousarray(
                outs[3][c * CL_SHARD[3]:(c + 1) * CL_SHARD[3]].T
                * S_W23).astype(BF16),
            "tgtidx": tix,
            "wm": wm,
            "tgtidx7": tix7,
            "wm7": wm7,
            "den": den_in,
            "rden": rden_in,
            "wgq": wgq_in,
        })
    return in_maps, S, S7


def _run(features, head_weight, proj0, out0, proj1, out1, proj2, out2,
         proj3, out3, discard_probs, targets, target_mask,
         trace=False, tmpdir=None):
    features = np.asarray(features, np.float32)
    head_weight = np.asarray(head_weight, np.float32)
    projs = [np.asarray(p, np.float32) for p in (proj0, proj1, proj2, proj3)]
    outs = [np.asarray(o, np.float32) for o in (out0, out1, out2, out3)]
    discard_probs = np.asarray(discard_probs, np.float32)

    in_maps, S, S7 = _shard_inputs(features, head_weight, projs, outs,
                                   discard_probs, targets, target_mask)
    nc = _build(S, S7)
    res = run_bass_kernel_spmd(nc, in_maps, list(range(NCORES)),
                               trace=trace, tmpdir=tmpdir)
    val = np.asarray(res.results[0]["out"], np.float32).reshape(())
    return val, res


def kernel(**inputs) -> np.ndarray:
    val, _ = _run(**inputs)
    return val
